# revision 1
# baseline (speedup 1.0000x reference)
"""GCN edge-aggregation kernel for 8 Trainium2 NeuronCores.

Math (see nn_GCNEdge): h = relu((segment_sum(edge_data, dst) / max(count,1)) @ W.T + b)

Strategy
--------
Host-side (sharding/layout only — all arithmetic happens on device):
  * Nodes are split contiguously across the 8 cores (12544 = 98 blocks of 128
    nodes per core; 8*12544 = 100352 >= 100000).
  * Each edge is routed to the core/block owning its destination node (CSR-style
    destination binning).  Within a block, edges occupy sequential slots; each
    block is padded to K_CHUNKS*128 slots so the device program is data-independent.
  * Edge features are shipped as a bf16 hi/lo pair (hi = bf16(x),
    lo = bf16(x - hi)) so the on-device f32-accumulated matmuls reconstruct
    ~fp32 precision while streaming at bf16 rates.  A constant-1 column rides
    along for the degree counts.

Device-side (per core, per 128-node block):
  * One-hot matrix of local node ids (DVE is_equal against an iota row),
  * PE matmul-accumulate onehot.T @ [x_hi | 1 | x_lo | 0] into PSUM -> per-node
    feature sums (hi+lo parts) and counts,
  * mean = sums * reciprocal(max(count, 1)),
  * PE transpose, then out = relu(W @ agg.T + b) via a second matmul with the
    (pre-transposed) weight as the stationary operand; output stays transposed
    [out_feat, node] and is un-transposed on the host.

No collectives are needed: output shards are disjoint.
"""

import numpy as np
import ml_dtypes

BF16 = ml_dtypes.bfloat16

N_NODES = 100000
N_EDGES = 1600000
F = 128
N_CORES = 8
BLK = 128                       # nodes per block
BLOCKS_PER_CORE = 98
TOTAL_BLOCKS = N_CORES * BLOCKS_PER_CORE        # 784
NODES_PER_CORE = BLOCKS_PER_CORE * BLK          # 12544
K_CHUNKS = 18                   # 128-edge chunks per block (capacity 2304 edges)

_module_cache = {}


def _build_module(K):
    import concourse.mybir as mybir
    import concourse.tile as tile
    from concourse import bacc

    f32 = mybir.dt.float32
    bf16 = mybir.dt.bfloat16
    RB = K * 128                 # edge slots per block
    SLOTS = BLOCKS_PER_CORE * RB

    nc = bacc.Bacc("TRN2", target_bir_lowering=False, debug=False)
    # xe rows are (block, partition); each row is that partition's K chunks of
    # 258 bf16 values laid contiguously -> 9KB-contiguous DMA descriptors.
    xe = nc.dram_tensor("xe", [BLOCKS_PER_CORE * 128, K * 258], bf16, kind="ExternalInput")
    lid = nc.dram_tensor("lid", [128, BLOCKS_PER_CORE * K], bf16, kind="ExternalInput")
    wt = nc.dram_tensor("wt", [128, 128], f32, kind="ExternalInput")
    bias = nc.dram_tensor("bias", [128, 1], f32, kind="ExternalInput")
    ident = nc.dram_tensor("ident", [128, 128], f32, kind="ExternalInput")
    # iota value pattern tiled K times: iotar[p, c*128 + f] = f
    iotar = nc.dram_tensor("iotar", [128, K * 128], bf16, kind="ExternalInput")
    out = nc.dram_tensor("out", [128, BLOCKS_PER_CORE * 128], f32, kind="ExternalOutput")

    xe_ap = xe.ap()
    out_ap = out.ap()

    with tile.TileContext(nc) as tc:
        with (
            tc.tile_pool(name="const", bufs=1) as cpool,
            tc.tile_pool(name="xp", bufs=6) as xpool,
            tc.tile_pool(name="ohp", bufs=8) as ohpool,
            tc.tile_pool(name="ep", bufs=3) as epool,
            tc.tile_pool(name="psS", bufs=4, space="PSUM") as psS,
            tc.tile_pool(name="psT", bufs=2, space="PSUM") as psT,
            tc.tile_pool(name="psO", bufs=2, space="PSUM") as psO,
        ):
            wt_t = cpool.tile([128, 128], f32)
            nc.sync.dma_start(wt_t[:], wt.ap()[:])
            bias_t = cpool.tile([128, 1], f32)
            nc.sync.dma_start(bias_t[:], bias.ap()[:])
            id_t = cpool.tile([128, 128], f32)
            nc.sync.dma_start(id_t[:], ident.ap()[:])
            iotar_t = cpool.tile([128, K * 128], bf16)
            nc.sync.dma_start(iotar_t[:], iotar.ap()[:])
            lid_t = cpool.tile([128, BLOCKS_PER_CORE * K], bf16)
            nc.sync.dma_start(lid_t[:], lid.ap()[:])

            group_pT = {}

            def emit_matmuls(b, xt, oh):
                ps = psS.tile([128, 258], f32, name=f"ps{b}", tag="ps")
                for c in range(K):
                    nc.tensor.matmul(
                        ps[:],
                        lhsT=oh[:, c * 128:(c + 1) * 128],
                        rhs=xt[:, c * 258:(c + 1) * 258],
                        start=(c == 0),
                        stop=(c == K - 1),
                    )
                return ps

            def emit_pscopy(b, ps):
                # Drain PSUM to SBUF with a single ACT copy (emitted one block
                # after the accumulation finished, so the ACT queue never
                # blocks on it) — frees the PSUM bank early; the lagged
                # epilogue then reads SBUF only.
                s_sb = epool.tile([128, 257], f32, name=f"s{b}", tag="s_sb", bufs=5)
                nc.scalar.copy(s_sb[:], ps[:, 0:257])
                return s_sb

            def emit_epilogue(b, ps):
                # counts live in ps[:,128] (the lo-side count column is all
                # zeros by construction), so no hi+lo add is needed for them.
                # No max(count,1) guard: the host guarantees every real node
                # has count > 0 (injecting 1e-30-weight phantom edges if
                # needed); padding nodes divide by zero -> NaN columns that
                # the host slices off.  Keeping DVE's per-block work to this
                # single tiny op is what lets the wide one-hot builds stream.
                rec = epool.tile([128, 1], f32, name=f"rec{b}", tag="rec")
                nc.vector.reciprocal(rec[:], ps[:, 128:129])
                # agg = (S_hi + S_lo)/count: t1 = S_hi*rec on ACT, then one
                # fused DVE op: agg = S_lo*rec + t1.
                t1 = epool.tile([128, 128], f32, name=f"t1{b}", tag="t1")
                nc.scalar.activation(
                    t1[:], ps[:, 0:128],
                    mybir.ActivationFunctionType.Copy, scale=rec[:, 0:1],
                )
                agg = epool.tile([128, 128], f32, name=f"agg{b}", tag="agg")
                nc.vector.scalar_tensor_tensor(
                    out=agg[:],
                    in0=ps[:, 129:257],
                    scalar=rec[:, 0:1],
                    in1=t1[:],
                    op0=mybir.AluOpType.mult,
                    op1=mybir.AluOpType.add,
                )
                # NOTE: `ps` here is the SBUF-staged copy (s_sb), not PSUM.
                j = b % 4
                if j == 0:
                    group_pT["t"] = psT.tile([128, 512], f32, name=f"pT{b}", tag="pT")
                pT = group_pT["t"]
                nc.tensor.transpose(pT[:, j * 128:(j + 1) * 128], agg[:], id_t[:])
                if j == 3 or b == BLOCKS_PER_CORE - 1:
                    g0 = (b // 4) * 4
                    gw = (b + 1 - g0) * 128
                    aggT = epool.tile([128, 512], f32, name=f"aggT{b}", tag="aggT", bufs=2)
                    nc.scalar.copy(aggT[:, 0:gw], pT[:, 0:gw])
                    pO = psO.tile([128, 512], f32, name=f"pO{b}", tag="pO")
                    nc.tensor.matmul(
                        pO[:, 0:gw], lhsT=wt_t[:], rhs=aggT[:, 0:gw],
                        start=True, stop=True,
                    )
                    ot = epool.tile([128, 512], f32, name=f"ot{b}", tag="ot", bufs=2)
                    nc.scalar.activation(
                        ot[:, 0:gw], pO[:, 0:gw],
                        mybir.ActivationFunctionType.Relu,
                        bias=bias_t[:, 0:1], scale=1.0,
                    )
                    nc.sync.dma_start(out_ap[:, g0 * 128:(b + 1) * 128], ot[:, 0:gw])

            # Software-pipelined emission. Every engine queue is strict
            # in-order, so an op gated on *fresh* upstream state stalls the
            # whole queue behind it. Stagger each stage so, by the time a
            # queue reaches an op, its dependencies are blocks old:
            #   iter b:  DMA xt(b) | one-hot TT(b) | PE matmuls(b-1)
            #            | PSUM->SBUF drain of (b-2) | epilogue of (b-5)
            pending = {}
            pending_ps = {}
            pending_s = {}
            for b in range(BLOCKS_PER_CORE):
                xt = xpool.tile([128, K * 258], bf16, name=f"xt{b}", tag="xt")
                nc.sync.dma_start(xt[:], xe_ap[b * 128:(b + 1) * 128, :])
                oh = ohpool.tile([128, K * 128], bf16, name=f"oh{b}", tag="oh")
                nc.vector.tensor_tensor(
                    out=oh[:].rearrange("p (c f) -> p c f", c=K),
                    in0=iotar_t[:].rearrange("p (c f) -> p c f", c=K),
                    in1=lid_t[:, b * K:(b + 1) * K].to_broadcast([128, K, 128]),
                    op=mybir.AluOpType.is_equal,
                )
                pending[b] = (xt, oh)
                if b >= 1:
                    pending_ps[b - 1] = emit_matmuls(b - 1, *pending.pop(b - 1))
                if b >= 2:
                    pending_s[b - 2] = emit_pscopy(b - 2, pending_ps.pop(b - 2))
                if b >= 5:
                    emit_epilogue(b - 5, pending_s.pop(b - 5))
            last = BLOCKS_PER_CORE - 1
            pending_ps[last] = emit_matmuls(last, *pending.pop(last))
            for bb in sorted(pending_ps):
                pending_s[bb] = emit_pscopy(bb, pending_ps.pop(bb))
            for bb in sorted(pending_s):
                emit_epilogue(bb, pending_s.pop(bb))

    nc.compile()
    return nc


def _get_module(K):
    if K not in _module_cache:
        _module_cache[K] = _build_module(K)
    return _module_cache[K]


def prepare_inputs(edge_data, dst, W, b):
    """Host-side sharding: route each edge to the core/block owning dst."""
    edge_data = np.asarray(edge_data, dtype=np.float32)
    dst = np.asarray(dst)
    W = np.asarray(W, dtype=np.float32)
    b = np.asarray(b, dtype=np.float32)
    E = dst.shape[0]

    # The device kernel divides by the raw count (no max(count,1) guard).
    # Give any zero-degree real node a phantom edge with zero features and a
    # 1e-30 "count" weight: sums stay exactly 0, so mean = 0/1e-30 = 0, which
    # matches the reference's 0/max(0,1).
    node_cnt = np.bincount(dst, minlength=N_NODES)[:N_NODES]
    zeros = np.nonzero(node_cnt == 0)[0]
    n_real = E
    if len(zeros):
        dst = np.concatenate([dst, zeros.astype(dst.dtype)])
        E = dst.shape[0]

    blk = (dst.astype(np.int64)) >> 7                 # destination block id
    cnt = np.bincount(blk, minlength=TOTAL_BLOCKS)
    K = max(K_CHUNKS, int(np.ceil(cnt.max() / 128)))
    RB = K * 128
    TOT = TOTAL_BLOCKS * RB

    starts = np.zeros(TOTAL_BLOCKS, np.int64)
    np.cumsum(cnt[:-1], out=starts[1:])
    order = np.argsort(blk, kind="stable")
    rank = np.empty(E, np.int64)
    rank[order] = np.arange(E, dtype=np.int64) - np.repeat(starts, cnt)
    slot = blk * RB + rank

    X = np.zeros((TOT, 258), BF16)
    xh = edge_data.astype(BF16)
    X[slot[:n_real], 0:128] = xh
    X[slot[:n_real], 128] = BF16(1.0)
    X[slot[:n_real], 129:257] = (edge_data - xh.astype(np.float32)).astype(BF16)
    if len(zeros):
        X[slot[n_real:], 128] = BF16(1e-30)
    # [block, chunk, partition, feat] -> [block, partition, chunk*feat] so each
    # SBUF partition's data is one long contiguous HBM run (big DMA descriptors).
    X = np.ascontiguousarray(
        X.reshape(TOTAL_BLOCKS, K, 128, 258).transpose(0, 2, 1, 3)
    ).reshape(N_CORES, BLOCKS_PER_CORE * 128, K * 258)

    lid_f = np.full(TOT, -1.0, np.float32)
    lid_f[slot] = (dst & 127).astype(np.float32)
    lid_all = (
        lid_f.reshape(N_CORES, BLOCKS_PER_CORE, K, 128)
        .transpose(0, 3, 1, 2)
        .reshape(N_CORES, 128, BLOCKS_PER_CORE * K)
        .astype(BF16)
    )
    wt = np.ascontiguousarray(W.T)
    bias = np.ascontiguousarray(b.reshape(128, 1))
    ident = np.eye(128, dtype=np.float32)
    iotar = np.ascontiguousarray(
        np.broadcast_to(
            np.arange(128, dtype=np.float32), (128, K, 128)
        ).reshape(128, K * 128)
    ).astype(BF16)

    in_maps = [
        {
            "xe": np.ascontiguousarray(X[c]),
            "lid": np.ascontiguousarray(lid_all[c]),
            "wt": wt,
            "bias": bias,
            "ident": ident,
            "iotar": iotar,
        }
        for c in range(N_CORES)
    ]
    return K, in_maps


def run(edge_data, dst, W, b, trace=False, tmpdir=None):
    from concourse.bass_utils import run_bass_kernel_spmd

    K, in_maps = prepare_inputs(edge_data, dst, W, b)
    nc = _get_module(K)
    res = run_bass_kernel_spmd(
        nc, in_maps, core_ids=list(range(N_CORES)), trace=trace, tmpdir=tmpdir,
    )
    outs = [res.results[c]["out"].T for c in range(N_CORES)]   # [12544, 128] each
    full = np.concatenate(outs, axis=0)[:N_NODES]
    return np.ascontiguousarray(full, dtype=np.float32), res


def kernel(edge_data, dst, W, b):
    out, _ = run(edge_data, dst, W, b, trace=False)
    return out



# revision 4
# speedup vs baseline: 1.5951x; 1.5951x over previous
"""GCN edge-aggregation kernel for 8 Trainium2 NeuronCores.

Math (see nn_GCNEdge): h = relu((segment_sum(edge_data, dst) / max(count,1)) @ W.T + b)

Strategy
--------
Host-side (sharding/layout only — all arithmetic happens on device):
  * Nodes are split contiguously across the 8 cores (12544 = 98 blocks of 128
    nodes per core; 8*12544 = 100352 >= 100000).
  * Each edge is routed to the core/block owning its destination node (CSR-style
    destination binning).  Within a block, edges occupy sequential slots; each
    block is padded to K_CHUNKS*128 slots so the device program is data-independent.
  * Edge features ship as plain bf16 (rel-err gate is 2e-2; bf16 end-to-end is
    ~3e-3), with a constant-1 count column riding along for the degree counts.

Device-side (per core, per 128-node block):
  * One-hot matrix of local node ids (DVE is_equal against an iota pattern).
    The one-hot is laid out [partition=edge, (node f, chunk c)] — f-major with
    the chunk axis innermost — so every DVE operand has a unit-stride last
    axis, which qualifies the op for the DVE 2x_1p fast path (2 elem/cycle).
    The PE matmul then reads each chunk's one-hot with a stride-K node axis.
  * PE matmul-accumulate onehot.T @ [x | 1 | 0] into PSUM -> per-node feature
    sums and counts,
  * mean = sums * reciprocal(count) (ACT copy with per-partition scale,
    casting to bf16),
  * PE transpose (bf16), then out = relu(W @ agg.T + b) via a bf16 matmul with
    the (pre-transposed) weight as the stationary operand; output stays
    transposed [out_feat, node] in bf16 and is un-transposed / upcast on host.

No collectives are needed: output shards are disjoint.
"""

import numpy as np
import ml_dtypes

BF16 = ml_dtypes.bfloat16

N_NODES = 100000
N_EDGES = 1600000
F = 128
N_CORES = 8
BLK = 128                       # nodes per block
BLOCKS_PER_CORE = 98
TOTAL_BLOCKS = N_CORES * BLOCKS_PER_CORE        # 784
NODES_PER_CORE = BLOCKS_PER_CORE * BLK          # 12544
K_CHUNKS = 18                   # 128-edge chunks per block (capacity 2304 edges)
XCOL = 130                      # 128 features + count col + pad col

_module_cache = {}


def _build_module(K):
    import concourse.mybir as mybir
    import concourse.tile as tile
    from concourse import bacc

    f32 = mybir.dt.float32
    bf16 = mybir.dt.bfloat16

    nc = bacc.Bacc("TRN2", target_bir_lowering=False, debug=False)
    # xe rows are (block, partition); each row is that partition's K chunks of
    # XCOL bf16 values laid contiguously -> 4.7KB-contiguous DMA descriptors.
    xe = nc.dram_tensor("xe", [BLOCKS_PER_CORE * 128, K * XCOL], bf16, kind="ExternalInput")
    lid = nc.dram_tensor("lid", [128, BLOCKS_PER_CORE * K], bf16, kind="ExternalInput")
    wt = nc.dram_tensor("wt", [128, 128], bf16, kind="ExternalInput")
    bias = nc.dram_tensor("bias", [128, 1], f32, kind="ExternalInput")
    ident = nc.dram_tensor("ident", [128, 128], bf16, kind="ExternalInput")
    # iota value pattern, f-major with chunk innermost: iotafc[p, f*K + c] = f
    iotafc = nc.dram_tensor("iotafc", [128, K * 128], bf16, kind="ExternalInput")
    out = nc.dram_tensor("out", [128, BLOCKS_PER_CORE * 128], bf16, kind="ExternalOutput")

    xe_ap = xe.ap()
    out_ap = out.ap()

    with tile.TileContext(nc) as tc:
        with (
            tc.tile_pool(name="const", bufs=1) as cpool,
            tc.tile_pool(name="xp", bufs=6) as xpool,
            tc.tile_pool(name="ohp", bufs=8) as ohpool,
            tc.tile_pool(name="ep", bufs=3) as epool,
            tc.tile_pool(name="psS", bufs=4, space="PSUM") as psS,
            tc.tile_pool(name="psT", bufs=2, space="PSUM") as psT,
            tc.tile_pool(name="psO", bufs=2, space="PSUM") as psO,
        ):
            wt_t = cpool.tile([128, 128], bf16)
            nc.sync.dma_start(wt_t[:], wt.ap()[:])
            bias_t = cpool.tile([128, 1], f32)
            nc.sync.dma_start(bias_t[:], bias.ap()[:])
            id_t = cpool.tile([128, 128], bf16)
            nc.sync.dma_start(id_t[:], ident.ap()[:])
            iotafc_t = cpool.tile([128, K * 128], bf16)
            nc.sync.dma_start(iotafc_t[:], iotafc.ap()[:])
            lid_t = cpool.tile([128, BLOCKS_PER_CORE * K], bf16)
            nc.sync.dma_start(lid_t[:], lid.ap()[:])

            group_pT = {}

            def emit_matmuls(b, xt, oh):
                ps = psS.tile([128, XCOL], f32, name=f"ps{b}", tag="ps")
                ohv = oh[:].rearrange("p (f c) -> p c f", c=K)
                for c in range(K):
                    nc.tensor.matmul(
                        ps[:],
                        lhsT=ohv[:, c, :],
                        rhs=xt[:, c * XCOL:(c + 1) * XCOL],
                        start=(c == 0),
                        stop=(c == K - 1),
                    )
                return ps

            def emit_scale(b, ps):
                # counts live in ps[:,128].  No max(count,1) guard: the host
                # guarantees every real node has count > 0 (injecting
                # 1e-30-weight phantom edges if needed); padding nodes divide
                # by zero -> NaN columns that the host slices off.
                rec = epool.tile([128, 1], f32, name=f"rec{b}", tag="rec")
                nc.vector.reciprocal(rec[:], ps[:, 128:129])
                agg = epool.tile([128, 128], bf16, name=f"agg{b}", tag="agg", bufs=5)
                nc.scalar.activation(
                    agg[:], ps[:, 0:128],
                    mybir.ActivationFunctionType.Copy, scale=rec[:, 0:1],
                )
                return agg

            def emit_tail(b, agg):
                j = b % 4
                if j == 0:
                    group_pT["t"] = psT.tile([128, 512], bf16, name=f"pT{b}", tag="pT")
                pT = group_pT["t"]
                nc.tensor.transpose(pT[:, j * 128:(j + 1) * 128], agg[:], id_t[:])
                if j == 3 or b == BLOCKS_PER_CORE - 1:
                    g0 = (b // 4) * 4
                    gw = (b + 1 - g0) * 128
                    aggT = epool.tile([128, 512], bf16, name=f"aggT{b}", tag="aggT", bufs=2)
                    nc.scalar.copy(aggT[:, 0:gw], pT[:, 0:gw])
                    pO = psO.tile([128, 512], f32, name=f"pO{b}", tag="pO")
                    nc.tensor.matmul(
                        pO[:, 0:gw], lhsT=wt_t[:], rhs=aggT[:, 0:gw],
                        start=True, stop=True,
                    )
                    ot = epool.tile([128, 512], bf16, name=f"ot{b}", tag="ot", bufs=2)
                    nc.scalar.activation(
                        ot[:, 0:gw], pO[:, 0:gw],
                        mybir.ActivationFunctionType.Relu,
                        bias=bias_t[:, 0:1], scale=1.0,
                    )
                    nc.sync.dma_start(out_ap[:, g0 * 128:(b + 1) * 128], ot[:, 0:gw])

            # Software-pipelined emission. Every engine queue is strict
            # in-order, so an op gated on *fresh* upstream state stalls the
            # whole queue behind it. Stagger each stage so, by the time a
            # queue reaches an op, its dependencies are blocks old:
            #   iter b:  DMA xt(b) | one-hot(b) | PE matmuls(b-1)
            #            | reciprocal+scale of (b-3) | transpose/output of (b-5)
            pending = {}
            pending_ps = {}
            pending_agg = {}
            for b in range(BLOCKS_PER_CORE):
                xt = xpool.tile([128, K * XCOL], bf16, name=f"xt{b}", tag="xt")
                nc.sync.dma_start(xt[:], xe_ap[b * 128:(b + 1) * 128, :])
                oh = ohpool.tile([128, K * 128], bf16, name=f"oh{b}", tag="oh")
                nc.vector.tensor_tensor(
                    out=oh[:].rearrange("p (f c) -> p f c", c=K),
                    in0=iotafc_t[:].rearrange("p (f c) -> p f c", c=K),
                    in1=lid_t[:, b * K:(b + 1) * K]
                        .rearrange("p (o c) -> p o c", o=1)
                        .to_broadcast([128, 128, K]),
                    op=mybir.AluOpType.is_equal,
                )
                pending[b] = (xt, oh)
                if b >= 1:
                    pending_ps[b - 1] = emit_matmuls(b - 1, *pending.pop(b - 1))
                if b >= 3:
                    pending_agg[b - 3] = emit_scale(b - 3, pending_ps.pop(b - 3))
                if b >= 5:
                    emit_tail(b - 5, pending_agg.pop(b - 5))
            last = BLOCKS_PER_CORE - 1
            pending_ps[last] = emit_matmuls(last, *pending.pop(last))
            for bb in sorted(pending_ps):
                pending_agg[bb] = emit_scale(bb, pending_ps.pop(bb))
            for bb in sorted(pending_agg):
                emit_tail(bb, pending_agg.pop(bb))

    nc.compile()
    return nc


def _get_module(K):
    if K not in _module_cache:
        _module_cache[K] = _build_module(K)
    return _module_cache[K]


def prepare_inputs(edge_data, dst, W, b):
    """Host-side sharding: route each edge to the core/block owning dst."""
    edge_data = np.asarray(edge_data, dtype=np.float32)
    dst = np.asarray(dst)
    W = np.asarray(W, dtype=np.float32)
    b = np.asarray(b, dtype=np.float32)
    E = dst.shape[0]

    # The device kernel divides by the raw count (no max(count,1) guard).
    # Give any zero-degree real node a phantom edge with zero features and a
    # 1e-30 "count" weight: sums stay exactly 0, so mean = 0/1e-30 = 0, which
    # matches the reference's 0/max(0,1).
    node_cnt = np.bincount(dst, minlength=N_NODES)[:N_NODES]
    zeros = np.nonzero(node_cnt == 0)[0]
    n_real = E
    if len(zeros):
        dst = np.concatenate([dst, zeros.astype(dst.dtype)])
        E = dst.shape[0]

    blk = (dst.astype(np.int64)) >> 7                 # destination block id
    cnt = np.bincount(blk, minlength=TOTAL_BLOCKS)
    K = max(K_CHUNKS, int(np.ceil(cnt.max() / 128)))
    RB = K * 128
    TOT = TOTAL_BLOCKS * RB

    starts = np.zeros(TOTAL_BLOCKS, np.int64)
    np.cumsum(cnt[:-1], out=starts[1:])
    order = np.argsort(blk, kind="stable")
    rank = np.empty(E, np.int64)
    rank[order] = np.arange(E, dtype=np.int64) - np.repeat(starts, cnt)
    slot = blk * RB + rank

    X = np.zeros((TOT, XCOL), BF16)
    X[slot[:n_real], 0:128] = edge_data.astype(BF16)
    X[slot[:n_real], 128] = BF16(1.0)
    if len(zeros):
        X[slot[n_real:], 128] = BF16(1e-30)
    # [block, chunk, partition, feat] -> [block, partition, chunk*feat] so each
    # SBUF partition's data is one long contiguous HBM run (big DMA descriptors).
    X = np.ascontiguousarray(
        X.reshape(TOTAL_BLOCKS, K, 128, XCOL).transpose(0, 2, 1, 3)
    ).reshape(N_CORES, BLOCKS_PER_CORE * 128, K * XCOL)

    lid_f = np.full(TOT, -1.0, np.float32)
    lid_f[slot] = (dst & 127).astype(np.float32)
    lid_all = (
        lid_f.reshape(N_CORES, BLOCKS_PER_CORE, K, 128)
        .transpose(0, 3, 1, 2)
        .reshape(N_CORES, 128, BLOCKS_PER_CORE * K)
        .astype(BF16)
    )
    wt = np.ascontiguousarray(W.T).astype(BF16)
    bias = np.ascontiguousarray(b.reshape(128, 1))
    ident = np.eye(128, dtype=np.float32).astype(BF16)
    # iotafc[p, f*K + c] = f
    iotafc = np.ascontiguousarray(
        np.broadcast_to(
            np.repeat(np.arange(128, dtype=np.float32), K), (128, K * 128)
        )
    ).astype(BF16)

    in_maps = [
        {
            "xe": np.ascontiguousarray(X[c]),
            "lid": np.ascontiguousarray(lid_all[c]),
            "wt": wt,
            "bias": bias,
            "ident": ident,
            "iotafc": iotafc,
        }
        for c in range(N_CORES)
    ]
    return K, in_maps


def run(edge_data, dst, W, b, trace=False, tmpdir=None):
    from concourse.bass_utils import run_bass_kernel_spmd

    K, in_maps = prepare_inputs(edge_data, dst, W, b)
    nc = _get_module(K)
    res = run_bass_kernel_spmd(
        nc, in_maps, core_ids=list(range(N_CORES)), trace=trace, tmpdir=tmpdir,
    )
    outs = [res.results[c]["out"].T for c in range(N_CORES)]   # [12544, 128] each
    full = np.concatenate(outs, axis=0)[:N_NODES]
    return np.ascontiguousarray(full, dtype=np.float32), res


def kernel(edge_data, dst, W, b):
    out, _ = run(edge_data, dst, W, b, trace=False)
    return out


# revision 10
# speedup vs baseline: 1.7992x; 1.1280x over previous
"""GCN edge-aggregation kernel for 8 Trainium2 NeuronCores.

Math (see nn_GCNEdge): h = relu((segment_sum(edge_data, dst) / max(count,1)) @ W.T + b)

Strategy
--------
Host-side (sharding/layout only — all arithmetic happens on device):
  * Nodes are split contiguously across the 8 cores (12544 = 98 blocks of 128
    nodes per core; 8*12544 = 100352 >= 100000).
  * Each edge is routed to the core/block owning its destination node (CSR-style
    destination binning).  Within a block, edges occupy sequential slots; each
    block is padded to K_CHUNKS*128 slots so the device program is data-independent.
  * Edge features ship as plain bf16 (rel-err gate is 2e-2; bf16 end-to-end is
    ~3e-3), with a constant-1 count column riding along for the degree counts.

Device-side (per core, per 128-node block):
  * One-hot matrix of local node ids (DVE is_equal against an iota pattern).
    The one-hot is laid out [partition=edge, (node f, chunk c)] — f-major with
    the chunk axis innermost — so every DVE operand has a unit-stride last
    axis, which qualifies the op for the DVE 2x_1p fast path (2 elem/cycle).
    The PE matmul then reads each chunk's one-hot with a stride-K node axis.
  * PE matmul-accumulate onehot.T @ [x | 1 | 0] into PSUM -> per-node feature
    sums and counts,
  * mean = sums * reciprocal(count) (ACT copy with per-partition scale,
    casting to bf16),
  * PE transpose (bf16), then out = relu(W @ agg.T + b) via a bf16 matmul with
    the (pre-transposed) weight as the stationary operand; output stays
    transposed [out_feat, node] in bf16 and is un-transposed / upcast on host.

No collectives are needed: output shards are disjoint.
"""

import numpy as np
import ml_dtypes

BF16 = ml_dtypes.bfloat16

N_NODES = 100000
N_EDGES = 1600000
F = 128
N_CORES = 8
BLK = 128                       # nodes per block
BLOCKS_PER_CORE = 98
TOTAL_BLOCKS = N_CORES * BLOCKS_PER_CORE        # 784
NODES_PER_CORE = BLOCKS_PER_CORE * BLK          # 12544
K_CHUNKS = 18                   # 128-edge chunks per block (capacity 2304 edges)
XCOL = 130                      # 128 features + count col + pad col

_module_cache = {}


def _build_module(K):
    import concourse.mybir as mybir
    import concourse.tile as tile
    from concourse import bacc

    f32 = mybir.dt.float32
    bf16 = mybir.dt.bfloat16

    nc = bacc.Bacc("TRN2", target_bir_lowering=False, debug=False)
    # xe is partition-major across all blocks: row p holds every block's K
    # chunks of XCOL bf16 values for partition p, so a multi-block DMA reads
    # GRP*4.7KB-contiguous runs per partition (big descriptors -> full HBM BW).
    xe = nc.dram_tensor("xe", [128, BLOCKS_PER_CORE * K * XCOL], bf16, kind="ExternalInput")
    lid = nc.dram_tensor("lid", [128, BLOCKS_PER_CORE * K], bf16, kind="ExternalInput")
    wt = nc.dram_tensor("wt", [128, 128], bf16, kind="ExternalInput")
    bias = nc.dram_tensor("bias", [128, 1], f32, kind="ExternalInput")
    ident = nc.dram_tensor("ident", [128, 128], bf16, kind="ExternalInput")
    # iota value pattern, f-major with chunk innermost: iotafc[p, f*K + c] = f
    iotafc = nc.dram_tensor("iotafc", [128, K * 128], bf16, kind="ExternalInput")
    out = nc.dram_tensor("out", [128, BLOCKS_PER_CORE * 128], bf16, kind="ExternalOutput")

    xe_ap = xe.ap()
    out_ap = out.ap()

    with tile.TileContext(nc) as tc:
        with (
            tc.tile_pool(name="const", bufs=1) as cpool,
            tc.tile_pool(name="xp", bufs=2) as xpool,
            tc.tile_pool(name="ohp", bufs=8) as ohpool,
            tc.tile_pool(name="ep", bufs=3) as epool,
            tc.tile_pool(name="psS", bufs=4, space="PSUM") as psS,
            tc.tile_pool(name="psT", bufs=2, space="PSUM") as psT,
            tc.tile_pool(name="psO", bufs=2, space="PSUM") as psO,
        ):
            # Constants ride the ACT engine's hardware DMA queue so the SP
            # queue is dedicated to the bulk xe stream.
            wt_t = cpool.tile([128, 128], bf16)
            nc.scalar.dma_start(wt_t[:], wt.ap()[:])
            bias_t = cpool.tile([128, 1], f32)
            nc.scalar.dma_start(bias_t[:], bias.ap()[:])
            id_t = cpool.tile([128, 128], bf16)
            nc.scalar.dma_start(id_t[:], ident.ap()[:])
            iotafc_t = cpool.tile([128, K * 128], bf16)
            nc.scalar.dma_start(iotafc_t[:], iotafc.ap()[:])
            lid_t = cpool.tile([128, BLOCKS_PER_CORE * K], bf16)
            nc.scalar.dma_start(lid_t[:], lid.ap()[:])

            group_pT = {}

            def emit_matmuls(b, xt, oh):
                ps = psS.tile([128, XCOL], f32, name=f"ps{b}", tag="ps")
                ohv = oh[:].rearrange("p (f c) -> p c f", c=K)
                for c in range(K):
                    nc.tensor.matmul(
                        ps[:],
                        lhsT=ohv[:, c, :],
                        rhs=xt[:, c * XCOL:(c + 1) * XCOL],
                        start=(c == 0),
                        stop=(c == K - 1),
                    )
                return ps

            def emit_scale(b, ps):
                # counts live in ps[:,128].  No max(count,1) guard: the host
                # guarantees every real node has count > 0 (injecting
                # 1e-30-weight phantom edges if needed); padding nodes divide
                # by zero -> NaN columns that the host slices off.
                rec = epool.tile([128, 1], f32, name=f"rec{b}", tag="rec")
                nc.vector.reciprocal(rec[:], ps[:, 128:129])
                agg = epool.tile([128, 128], bf16, name=f"agg{b}", tag="agg", bufs=5)
                nc.scalar.activation(
                    agg[:], ps[:, 0:128],
                    mybir.ActivationFunctionType.Copy, scale=rec[:, 0:1],
                )
                return agg

            def emit_tail(b, agg):
                j = b % 4
                if j == 0:
                    group_pT["t"] = psT.tile([128, 512], bf16, name=f"pT{b}", tag="pT")
                pT = group_pT["t"]
                nc.tensor.transpose(pT[:, j * 128:(j + 1) * 128], agg[:], id_t[:])
                if j == 3 or b == BLOCKS_PER_CORE - 1:
                    g0 = (b // 4) * 4
                    gw = (b + 1 - g0) * 128
                    aggT = epool.tile([128, 512], bf16, name=f"aggT{b}", tag="aggT", bufs=2)
                    nc.scalar.copy(aggT[:, 0:gw], pT[:, 0:gw])
                    pO = psO.tile([128, 512], f32, name=f"pO{b}", tag="pO")
                    nc.tensor.matmul(
                        pO[:, 0:gw], lhsT=wt_t[:], rhs=aggT[:, 0:gw],
                        start=True, stop=True,
                    )
                    ot = epool.tile([128, 512], bf16, name=f"ot{b}", tag="ot", bufs=2)
                    nc.scalar.activation(
                        ot[:, 0:gw], pO[:, 0:gw],
                        mybir.ActivationFunctionType.Relu,
                        bias=bias_t[:, 0:1], scale=1.0,
                    )
                    nc.scalar.dma_start(out_ap[:, g0 * 128:(b + 1) * 128], ot[:, 0:gw])

            # Software-pipelined emission. Every engine queue is strict
            # in-order, so an op gated on *fresh* upstream state stalls the
            # whole queue behind it. Stagger each stage so, by the time a
            # queue reaches an op, its dependencies are blocks old:
            #   iter b:  DMA xt(b) | one-hot(b) | PE matmuls(b-1)
            #            | reciprocal+scale of (b-3) | transpose/output of (b-5)
            GRP = 7            # blocks per xe DMA (98 = 14 * 7): 33KB/partition runs
            BCOL = K * XCOL
            xt_groups = {}
            pending = {}
            pending_ps = {}
            pending_agg = {}
            for b in range(BLOCKS_PER_CORE):
                g, j = divmod(b, GRP)
                if j == 0:
                    xg = xpool.tile([128, GRP * BCOL], bf16, name=f"xg{g}", tag="xg")
                    nc.sync.dma_start(
                        xg[:], xe_ap[:, g * GRP * BCOL:(g + 1) * GRP * BCOL]
                    )
                    xt_groups[g] = xg
                xt = xt_groups[g][:, j * BCOL:(j + 1) * BCOL]
                oh = ohpool.tile([128, K * 128], bf16, name=f"oh{b}", tag="oh")
                nc.vector.tensor_tensor(
                    out=oh[:].rearrange("p (f c) -> p f c", c=K),
                    in0=iotafc_t[:].rearrange("p (f c) -> p f c", c=K),
                    in1=lid_t[:, b * K:(b + 1) * K]
                        .rearrange("p (o c) -> p o c", o=1)
                        .to_broadcast([128, 128, K]),
                    op=mybir.AluOpType.is_equal,
                )
                pending[b] = (xt, oh)
                if b >= 1:
                    pending_ps[b - 1] = emit_matmuls(b - 1, *pending.pop(b - 1))
                if b >= 3:
                    pending_agg[b - 3] = emit_scale(b - 3, pending_ps.pop(b - 3))
                if b >= 5:
                    emit_tail(b - 5, pending_agg.pop(b - 5))
            last = BLOCKS_PER_CORE - 1
            pending_ps[last] = emit_matmuls(last, *pending.pop(last))
            for bb in sorted(pending_ps):
                pending_agg[bb] = emit_scale(bb, pending_ps.pop(bb))
            for bb in sorted(pending_agg):
                emit_tail(bb, pending_agg.pop(bb))

    nc.compile()
    return nc


def _get_module(K):
    if K not in _module_cache:
        _module_cache[K] = _build_module(K)
    return _module_cache[K]


def prepare_inputs(edge_data, dst, W, b):
    """Host-side sharding: route each edge to the core/block owning dst."""
    edge_data = np.asarray(edge_data, dtype=np.float32)
    dst = np.asarray(dst)
    W = np.asarray(W, dtype=np.float32)
    b = np.asarray(b, dtype=np.float32)
    E = dst.shape[0]

    # The device kernel divides by the raw count (no max(count,1) guard).
    # Give any zero-degree real node a phantom edge with zero features and a
    # 1e-30 "count" weight: sums stay exactly 0, so mean = 0/1e-30 = 0, which
    # matches the reference's 0/max(0,1).
    node_cnt = np.bincount(dst, minlength=N_NODES)[:N_NODES]
    zeros = np.nonzero(node_cnt == 0)[0]
    n_real = E
    if len(zeros):
        dst = np.concatenate([dst, zeros.astype(dst.dtype)])
        E = dst.shape[0]

    blk = (dst.astype(np.int64)) >> 7                 # destination block id
    cnt = np.bincount(blk, minlength=TOTAL_BLOCKS)
    K = max(K_CHUNKS, int(np.ceil(cnt.max() / 128)))
    RB = K * 128
    TOT = TOTAL_BLOCKS * RB

    starts = np.zeros(TOTAL_BLOCKS, np.int64)
    np.cumsum(cnt[:-1], out=starts[1:])
    order = np.argsort(blk, kind="stable")
    rank = np.empty(E, np.int64)
    rank[order] = np.arange(E, dtype=np.int64) - np.repeat(starts, cnt)
    slot = blk * RB + rank

    X = np.zeros((TOT, XCOL), BF16)
    X[slot[:n_real], 0:128] = edge_data.astype(BF16)
    X[slot[:n_real], 128] = BF16(1.0)
    if len(zeros):
        X[slot[n_real:], 128] = BF16(1e-30)
    # [core, block, chunk, partition, feat] -> [core, partition, block*chunk*feat]
    # (partition-major) so each SBUF partition's data for a run of blocks is one
    # long contiguous HBM range (33KB DMA descriptors at 7 blocks per transfer).
    X = np.ascontiguousarray(
        X.reshape(N_CORES, BLOCKS_PER_CORE, K, 128, XCOL).transpose(0, 3, 1, 2, 4)
    ).reshape(N_CORES, 128, BLOCKS_PER_CORE * K * XCOL)

    lid_f = np.full(TOT, -1.0, np.float32)
    lid_f[slot] = (dst & 127).astype(np.float32)
    lid_all = (
        lid_f.reshape(N_CORES, BLOCKS_PER_CORE, K, 128)
        .transpose(0, 3, 1, 2)
        .reshape(N_CORES, 128, BLOCKS_PER_CORE * K)
        .astype(BF16)
    )
    wt = np.ascontiguousarray(W.T).astype(BF16)
    bias = np.ascontiguousarray(b.reshape(128, 1))
    ident = np.eye(128, dtype=np.float32).astype(BF16)
    # iotafc[p, f*K + c] = f
    iotafc = np.ascontiguousarray(
        np.broadcast_to(
            np.repeat(np.arange(128, dtype=np.float32), K), (128, K * 128)
        )
    ).astype(BF16)

    in_maps = [
        {
            "xe": np.ascontiguousarray(X[c]),
            "lid": np.ascontiguousarray(lid_all[c]),
            "wt": wt,
            "bias": bias,
            "ident": ident,
            "iotafc": iotafc,
        }
        for c in range(N_CORES)
    ]
    return K, in_maps


def run(edge_data, dst, W, b, trace=False, tmpdir=None):
    from concourse.bass_utils import run_bass_kernel_spmd

    K, in_maps = prepare_inputs(edge_data, dst, W, b)
    nc = _get_module(K)
    res = run_bass_kernel_spmd(
        nc, in_maps, core_ids=list(range(N_CORES)), trace=trace, tmpdir=tmpdir,
    )
    outs = [res.results[c]["out"].T for c in range(N_CORES)]   # [12544, 128] each
    full = np.concatenate(outs, axis=0)[:N_NODES]
    return np.ascontiguousarray(full, dtype=np.float32), res


def kernel(edge_data, dst, W, b):
    out, _ = run(edge_data, dst, W, b, trace=False)
    return out


# revision 11
# speedup vs baseline: 1.8068x; 1.0042x over previous
"""GCN edge-aggregation kernel for 8 Trainium2 NeuronCores.

Math (see nn_GCNEdge): h = relu((segment_sum(edge_data, dst) / max(count,1)) @ W.T + b)

Strategy
--------
Host-side (sharding/layout only — all arithmetic happens on device):
  * Nodes live in 784 blocks of 128; each edge is routed to the block owning
    its destination node (CSR-style destination binning).  Blocks are
    bin-packed onto the 8 cores: sorted by chunk count (ceil(edges/128)) and
    dealt round-robin, so every core sees the same per-position chunk-count
    sequence kb_seq and one SPMD program serves all cores, with only ~4% slot
    padding (vs ~13% for a uniform 18-chunk capacity).
  * Edge features ship as plain bf16 (rel-err gate is 2e-2; bf16 end-to-end is
    ~3e-3), with a constant-1 count column riding along for the degree counts.
  * The xe stream is partition-major: each SBUF partition's data for a run of
    blocks is one contiguous HBM range, so multi-block DMAs move ~30KB per
    partition per transfer (big descriptors -> full HBM bandwidth).

Device-side (per core, per 128-node block):
  * One-hot matrix of local node ids (DVE is_equal against an iota pattern).
    The one-hot is laid out [partition=edge, (node f, chunk c)] — f-major with
    the chunk axis innermost — so every DVE operand has a unit-stride last
    axis, which qualifies the op for the DVE 2x_1p fast path (2 elem/cycle).
    The PE matmul then reads each chunk's one-hot with a stride-K node axis.
  * PE matmul-accumulate onehot.T @ [x | 1 | 0] into PSUM -> per-node feature
    sums and counts,
  * mean = sums * reciprocal(count) (ACT copy with per-partition scale,
    casting to bf16),
  * PE transpose (bf16), then out = relu(W @ agg.T + b) via a bf16 matmul with
    the (pre-transposed) weight as the stationary operand; output stays
    transposed [out_feat, node] in bf16 and is un-transposed / upcast on host.

No collectives are needed: output shards are disjoint.
"""

import numpy as np
import ml_dtypes

BF16 = ml_dtypes.bfloat16

N_NODES = 100000
N_EDGES = 1600000
F = 128
N_CORES = 8
BLK = 128                       # nodes per block
BLOCKS_PER_CORE = 98
TOTAL_BLOCKS = N_CORES * BLOCKS_PER_CORE        # 784
NODES_PER_CORE = BLOCKS_PER_CORE * BLK          # 12544
XCOL = 130                      # 128 features + count col + pad col

_module_cache = {}


def _make_groups():
    """Positions per xe DMA transfer: big groups for bandwidth, tapered tail
    so the final blocks' compute overlaps the last transfers."""
    return [7] * 13 + [4, 2, 1]


def _build_module(kb_seq):
    import concourse.mybir as mybir
    import concourse.tile as tile
    from concourse import bacc

    f32 = mybir.dt.float32
    bf16 = mybir.dt.bfloat16

    kb_seq = list(kb_seq)
    nblocks = len(kb_seq)
    CH = sum(kb_seq)                       # total chunks per core
    prefix = np.concatenate([[0], np.cumsum(kb_seq)]).astype(int)
    kdistinct = sorted(set(kb_seq))
    ioff = {}
    o = 0
    for k in kdistinct:
        ioff[k] = o
        o += k * 128
    IOTA_COLS = o
    KMAX = max(kb_seq)

    groups = _make_groups()
    assert sum(groups) == nblocks
    gstart = np.concatenate([[0], np.cumsum(groups)]).astype(int)
    GMAXCH = max(
        prefix[gstart[gi + 1]] - prefix[gstart[gi]] for gi in range(len(groups))
    )

    nc = bacc.Bacc("TRN2", target_bir_lowering=False, debug=False)
    xe = nc.dram_tensor("xe", [128, CH * XCOL], bf16, kind="ExternalInput")
    lid = nc.dram_tensor("lid", [128, CH], bf16, kind="ExternalInput")
    wt = nc.dram_tensor("wt", [128, 128], bf16, kind="ExternalInput")
    bias = nc.dram_tensor("bias", [128, 1], f32, kind="ExternalInput")
    ident = nc.dram_tensor("ident", [128, 128], bf16, kind="ExternalInput")
    # iota value patterns, f-major with chunk innermost, one per distinct kb:
    # iotafc[p, ioff[k] + f*k + c] = f
    iotafc = nc.dram_tensor("iotafc", [128, IOTA_COLS], bf16, kind="ExternalInput")
    out = nc.dram_tensor("out", [128, nblocks * 128], bf16, kind="ExternalOutput")

    xe_ap = xe.ap()
    out_ap = out.ap()

    with tile.TileContext(nc) as tc:
        with (
            tc.tile_pool(name="const", bufs=1) as cpool,
            tc.tile_pool(name="xp", bufs=3) as xpool,
            tc.tile_pool(name="ohp", bufs=8) as ohpool,
            tc.tile_pool(name="ep", bufs=3) as epool,
            tc.tile_pool(name="psS", bufs=4, space="PSUM") as psS,
            tc.tile_pool(name="psT", bufs=2, space="PSUM") as psT,
            tc.tile_pool(name="psO", bufs=2, space="PSUM") as psO,
        ):
            # Constants ride the ACT engine's hardware DMA queue so the SP
            # queue is dedicated to the bulk xe stream.
            wt_t = cpool.tile([128, 128], bf16)
            nc.scalar.dma_start(wt_t[:], wt.ap()[:])
            bias_t = cpool.tile([128, 1], f32)
            nc.scalar.dma_start(bias_t[:], bias.ap()[:])
            id_t = cpool.tile([128, 128], bf16)
            nc.scalar.dma_start(id_t[:], ident.ap()[:])
            iotafc_t = cpool.tile([128, IOTA_COLS], bf16)
            nc.scalar.dma_start(iotafc_t[:], iotafc.ap()[:])
            lid_t = cpool.tile([128, CH], bf16)
            nc.scalar.dma_start(lid_t[:], lid.ap()[:])

            group_pT = {}

            def emit_matmuls(b, xt, oh):
                kb = kb_seq[b]
                ps = psS.tile([128, XCOL], f32, name=f"ps{b}", tag="ps")
                ohv = oh[:, 0:kb * 128].rearrange("p (f c) -> p c f", c=kb)
                for c in range(kb):
                    nc.tensor.matmul(
                        ps[:],
                        lhsT=ohv[:, c, :],
                        rhs=xt[:, c * XCOL:(c + 1) * XCOL],
                        start=(c == 0),
                        stop=(c == kb - 1),
                    )
                return ps

            def emit_scale(b, ps):
                # counts live in ps[:,128].  No max(count,1) guard: the host
                # guarantees every real node has count > 0 (injecting
                # 1e-30-weight phantom edges if needed); padding nodes divide
                # by zero -> NaN columns that the host slices off.
                rec = epool.tile([128, 1], f32, name=f"rec{b}", tag="rec")
                nc.vector.reciprocal(rec[:], ps[:, 128:129])
                agg = epool.tile([128, 128], bf16, name=f"agg{b}", tag="agg", bufs=5)
                nc.scalar.activation(
                    agg[:], ps[:, 0:128],
                    mybir.ActivationFunctionType.Copy, scale=rec[:, 0:1],
                )
                return agg

            def emit_tail(b, agg):
                j = b % 4
                if j == 0:
                    group_pT["t"] = psT.tile([128, 512], bf16, name=f"pT{b}", tag="pT")
                pT = group_pT["t"]
                nc.tensor.transpose(pT[:, j * 128:(j + 1) * 128], agg[:], id_t[:])
                if j == 3 or b == nblocks - 1:
                    g0 = (b // 4) * 4
                    gw = (b + 1 - g0) * 128
                    aggT = epool.tile([128, 512], bf16, name=f"aggT{b}", tag="aggT", bufs=2)
                    nc.scalar.copy(aggT[:, 0:gw], pT[:, 0:gw])
                    pO = psO.tile([128, 512], f32, name=f"pO{b}", tag="pO")
                    nc.tensor.matmul(
                        pO[:, 0:gw], lhsT=wt_t[:], rhs=aggT[:, 0:gw],
                        start=True, stop=True,
                    )
                    ot = epool.tile([128, 512], bf16, name=f"ot{b}", tag="ot", bufs=2)
                    nc.scalar.activation(
                        ot[:, 0:gw], pO[:, 0:gw],
                        mybir.ActivationFunctionType.Relu,
                        bias=bias_t[:, 0:1], scale=1.0,
                    )
                    nc.scalar.dma_start(out_ap[:, g0 * 128:(b + 1) * 128], ot[:, 0:gw])

            # Software-pipelined emission. Every engine queue is strict
            # in-order, so an op gated on *fresh* upstream state stalls the
            # whole queue behind it. Stagger each stage so, by the time a
            # queue reaches an op, its dependencies are blocks old:
            #   iter b:  DMA xe group | one-hot(b) | PE matmuls(b-1)
            #            | reciprocal+scale of (b-3) | transpose/output of (b-5)
            xt_of = {}
            gi = 0
            pending = {}
            pending_ps = {}
            pending_agg = {}
            for b in range(nblocks):
                if gi < len(groups) and b == gstart[gi]:
                    c0, c1 = prefix[gstart[gi]], prefix[gstart[gi + 1]]
                    xg = xpool.tile([128, GMAXCH * XCOL], bf16, name=f"xg{gi}", tag="xg")
                    nc.sync.dma_start(
                        xg[:, 0:(c1 - c0) * XCOL],
                        xe_ap[:, c0 * XCOL:c1 * XCOL],
                    )
                    for bb in range(gstart[gi], gstart[gi + 1]):
                        off = (prefix[bb] - c0) * XCOL
                        xt_of[bb] = xg[:, off:off + kb_seq[bb] * XCOL]
                    gi += 1
                kb = kb_seq[b]
                oh = ohpool.tile([128, KMAX * 128], bf16, name=f"oh{b}", tag="oh")
                nc.vector.tensor_tensor(
                    out=oh[:, 0:kb * 128].rearrange("p (f c) -> p f c", c=kb),
                    in0=iotafc_t[:, ioff[kb]:ioff[kb] + kb * 128]
                        .rearrange("p (f c) -> p f c", c=kb),
                    in1=lid_t[:, prefix[b]:prefix[b] + kb]
                        .rearrange("p (o c) -> p o c", o=1)
                        .to_broadcast([128, 128, kb]),
                    op=mybir.AluOpType.is_equal,
                )
                pending[b] = oh
                if b >= 1:
                    bb = b - 1
                    pending_ps[bb] = emit_matmuls(bb, xt_of.pop(bb), pending.pop(bb))
                if b >= 3:
                    pending_agg[b - 3] = emit_scale(b - 3, pending_ps.pop(b - 3))
                if b >= 5:
                    emit_tail(b - 5, pending_agg.pop(b - 5))
            last = nblocks - 1
            pending_ps[last] = emit_matmuls(last, xt_of.pop(last), pending.pop(last))
            for bb in sorted(pending_ps):
                pending_agg[bb] = emit_scale(bb, pending_ps.pop(bb))
            for bb in sorted(pending_agg):
                emit_tail(bb, pending_agg.pop(bb))

    nc.compile()
    return nc


def _get_module(kb_seq):
    key = tuple(kb_seq)
    if key not in _module_cache:
        _module_cache[key] = _build_module(key)
    return _module_cache[key]


def prepare_inputs(edge_data, dst, W, b):
    """Host-side sharding: route each edge to the core/block owning dst."""
    edge_data = np.asarray(edge_data, dtype=np.float32)
    dst = np.asarray(dst)
    W = np.asarray(W, dtype=np.float32)
    b = np.asarray(b, dtype=np.float32)
    E = dst.shape[0]

    # The device kernel divides by the raw count (no max(count,1) guard).
    # Give any zero-degree real node a phantom edge with zero features and a
    # 1e-30 "count" weight: sums stay exactly 0, so mean = 0/1e-30 = 0, which
    # matches the reference's 0/max(0,1).
    node_cnt = np.bincount(dst, minlength=N_NODES)[:N_NODES]
    zeros = np.nonzero(node_cnt == 0)[0]
    n_real = E
    if len(zeros):
        dst = np.concatenate([dst, zeros.astype(dst.dtype)])
        E = dst.shape[0]

    blk = (dst.astype(np.int64)) >> 7                 # destination block id
    cnt = np.bincount(blk, minlength=TOTAL_BLOCKS)
    kb_all = np.maximum(1, -(-cnt // 128))            # chunks per block

    # Bin-pack: sort blocks by chunk count desc, deal round-robin to cores.
    # Every core then has the same chunk-count sequence kb_seq (per-position
    # max over cores = the first core's, since the deal preserves order).
    sortidx = np.argsort(-kb_all, kind="stable")
    core_of = np.empty(TOTAL_BLOCKS, np.int64)
    pos_of = np.empty(TOTAL_BLOCKS, np.int64)
    r = np.arange(TOTAL_BLOCKS)
    core_of[sortidx] = r % N_CORES
    pos_of[sortidx] = r // N_CORES
    kb_seq = kb_all[sortidx[0::N_CORES]]
    CH = int(kb_seq.sum())
    prefix = np.concatenate([[0], np.cumsum(kb_seq)]).astype(np.int64)

    starts = np.zeros(TOTAL_BLOCKS, np.int64)
    np.cumsum(cnt[:-1], out=starts[1:])
    order = np.argsort(blk, kind="stable")
    rank = np.empty(E, np.int64)
    rank[order] = np.arange(E, dtype=np.int64) - np.repeat(starts, cnt)

    # Flat slot in the per-core partition-major layout:
    #   (core*128 + partition) * CH + prefix[pos] + chunk
    slot = (
        (core_of[blk] * 128 + (rank & 127)) * CH
        + prefix[pos_of[blk]] + (rank >> 7)
    )

    X = np.zeros((N_CORES * 128 * CH, XCOL), BF16)
    X[slot[:n_real], 0:128] = edge_data.astype(BF16)
    X[slot[:n_real], 128] = BF16(1.0)
    if len(zeros):
        X[slot[n_real:], 128] = BF16(1e-30)
    X = X.reshape(N_CORES, 128, CH * XCOL)

    lid_f = np.full(N_CORES * 128 * CH, -1.0, np.float32)
    lid_f[slot] = (dst & 127).astype(np.float32)
    lid_all = lid_f.reshape(N_CORES, 128, CH).astype(BF16)

    wt = np.ascontiguousarray(W.T).astype(BF16)
    bias = np.ascontiguousarray(b.reshape(128, 1))
    ident = np.eye(128, dtype=np.float32).astype(BF16)
    kdistinct = sorted(set(int(k) for k in kb_seq))
    iotafc = np.concatenate(
        [np.repeat(np.arange(128, dtype=np.float32), k) for k in kdistinct]
    )
    iotafc = np.ascontiguousarray(
        np.broadcast_to(iotafc, (128, iotafc.shape[0]))
    ).astype(BF16)

    in_maps = [
        {
            "xe": np.ascontiguousarray(X[c]),
            "lid": np.ascontiguousarray(lid_all[c]),
            "wt": wt,
            "bias": bias,
            "ident": ident,
            "iotafc": iotafc,
        }
        for c in range(N_CORES)
    ]
    return kb_seq, sortidx, in_maps


def run(edge_data, dst, W, b, trace=False, tmpdir=None):
    from concourse.bass_utils import run_bass_kernel_spmd

    kb_seq, sortidx, in_maps = prepare_inputs(edge_data, dst, W, b)
    nc = _get_module(kb_seq)
    res = run_bass_kernel_spmd(
        nc, in_maps, core_ids=list(range(N_CORES)), trace=trace, tmpdir=tmpdir,
    )
    full = np.empty((TOTAL_BLOCKS * 128, 128), np.float32)
    for c in range(N_CORES):
        oc = res.results[c]["out"].T.astype(np.float32)   # [12544, 128]
        blocks = sortidx[c::N_CORES]                      # block at position j
        for j, blkid in enumerate(blocks):
            full[blkid * 128:(blkid + 1) * 128] = oc[j * 128:(j + 1) * 128]
    full = full[:N_NODES]
    return np.ascontiguousarray(full, dtype=np.float32), res


def kernel(edge_data, dst, W, b):
    out, _ = run(edge_data, dst, W, b, trace=False)
    return out


# revision 18
# speedup vs baseline: 2.2373x; 1.2382x over previous
"""GCN edge-aggregation kernel for 8 Trainium2 NeuronCores.

Math (see nn_GCNEdge): h = relu((segment_sum(edge_data, dst) / max(count,1)) @ W.T + b)

Strategy
--------
Host-side (sharding/layout only — all arithmetic happens on device):
  * Nodes live in 784 blocks of 128; each edge is routed to the block owning
    its destination node (CSR-style destination binning).  Blocks are
    bin-packed onto the 8 cores: sorted by chunk count (ceil(edges/128)) and
    dealt round-robin, so every core sees the same per-position chunk-count
    sequence kb_seq and one SPMD program serves all cores, with only ~4% slot
    padding (vs ~13% for a uniform 18-chunk capacity).
  * Edge features ship as plain bf16 (rel-err gate is 2e-2; bf16 end-to-end is
    ~3e-3), with a constant-1 count column riding along for the degree counts.
  * The xe stream is partition-major: each SBUF partition's data for a run of
    blocks is one contiguous HBM range, so multi-block DMAs move ~30KB per
    partition per transfer (big descriptors -> full HBM bandwidth).

Device-side (per core, per 128-node block):
  * One-hot matrix of local node ids (DVE is_equal against an iota pattern).
    The one-hot is laid out [partition=edge, (node f, chunk c)] — f-major with
    the chunk axis innermost — so every DVE operand has a unit-stride last
    axis, which qualifies the op for the DVE 2x_1p fast path (2 elem/cycle).
    The PE matmul then reads each chunk's one-hot with a stride-K node axis.
  * PE matmul-accumulate onehot.T @ [x | 1 | 0] into PSUM -> per-node feature
    sums and counts,
  * mean = sums * reciprocal(count) (ACT copy with per-partition scale,
    casting to bf16),
  * PE transpose (bf16), then out = relu(W @ agg.T + b) via a bf16 matmul with
    the (pre-transposed) weight as the stationary operand; output stays
    transposed [out_feat, node] in bf16 and is un-transposed / upcast on host.

No collectives are needed: output shards are disjoint.
"""

import numpy as np
import ml_dtypes

BF16 = ml_dtypes.bfloat16

N_NODES = 100000
N_EDGES = 1600000
F = 128
N_CORES = 8
BLK = 128                       # nodes per block
BLOCKS_PER_CORE = 98
TOTAL_BLOCKS = N_CORES * BLOCKS_PER_CORE        # 784
NODES_PER_CORE = BLOCKS_PER_CORE * BLK          # 12544
XCOL = 129                      # 128 features + count col

_module_cache = {}


def _make_groups():
    """Positions per xe DMA transfer: tapered head so compute starts early,
    big groups in the middle for bandwidth, tapered tail so the final blocks'
    compute overlaps the last transfers."""
    return [2, 2, 3, 4, 5, 6] + [7] * 10 + [2, 2, 1, 1]


def _build_module(kb_seq):
    import concourse.mybir as mybir
    import concourse.tile as tile
    from concourse import bacc

    f32 = mybir.dt.float32
    bf16 = mybir.dt.bfloat16

    kb_seq = list(kb_seq)
    nblocks = len(kb_seq)
    CH = sum(kb_seq)                       # total chunks per core
    prefix = np.concatenate([[0], np.cumsum(kb_seq)]).astype(int)
    kdistinct = sorted(set(kb_seq))
    ioff = {}
    o = 0
    for k in kdistinct:
        ioff[k] = o
        o += k * 128
    IOTA_COLS = o
    KMAX = max(kb_seq)

    groups = _make_groups()
    assert sum(groups) == nblocks
    gstart = np.concatenate([[0], np.cumsum(groups)]).astype(int)
    GMAXCH = max(
        prefix[gstart[gi + 1]] - prefix[gstart[gi]] for gi in range(len(groups))
    )

    nc = bacc.Bacc("TRN2", target_bir_lowering=False, debug=False)
    xe = nc.dram_tensor("xe", [128, CH * XCOL], bf16, kind="ExternalInput")
    lid = nc.dram_tensor("lid", [128, CH], bf16, kind="ExternalInput")
    wt = nc.dram_tensor("wt", [128, 128], bf16, kind="ExternalInput")
    bias = nc.dram_tensor("bias", [128, 1], f32, kind="ExternalInput")
    ident = nc.dram_tensor("ident", [128, 128], bf16, kind="ExternalInput")
    # iota value patterns, f-major with chunk innermost, one per distinct kb:
    # iotafc[p, ioff[k] + f*k + c] = f
    iotafc = nc.dram_tensor("iotafc", [128, IOTA_COLS], bf16, kind="ExternalInput")
    out = nc.dram_tensor("out", [128, nblocks * 128], bf16, kind="ExternalOutput")

    xe_ap = xe.ap()
    out_ap = out.ap()

    with tile.TileContext(nc) as tc:
        with (
            tc.tile_pool(name="const", bufs=1) as cpool,
            tc.tile_pool(name="xp", bufs=4) as xpool,
            tc.tile_pool(name="ohp", bufs=8) as ohpool,
            tc.tile_pool(name="ep", bufs=3) as epool,
            tc.tile_pool(name="psS", bufs=4, space="PSUM") as psS,
            tc.tile_pool(name="psT", bufs=2, space="PSUM") as psT,
            tc.tile_pool(name="psO", bufs=2, space="PSUM") as psO,
        ):
            # Constants ride the ACT engine's hardware DMA queue so the SP
            # queue starts the bulk xe stream immediately; the one-hot
            # prerequisites (iotafc, lid) load first.
            iotafc_t = cpool.tile([128, IOTA_COLS], bf16)
            nc.scalar.dma_start(iotafc_t[:], iotafc.ap()[:])
            lid_t = cpool.tile([128, CH], bf16)
            nc.scalar.dma_start(lid_t[:], lid.ap()[:])
            wt_t = cpool.tile([128, 128], bf16)
            nc.scalar.dma_start(wt_t[:], wt.ap()[:])
            bias_t = cpool.tile([128, 1], f32)
            nc.scalar.dma_start(bias_t[:], bias.ap()[:])
            id_t = cpool.tile([128, 128], bf16)
            nc.scalar.dma_start(id_t[:], ident.ap()[:])

            group_pT = {}

            def emit_matmuls(b, xt, oh):
                kb = kb_seq[b]
                ps = psS.tile([128, XCOL], f32, name=f"ps{b}", tag="ps")
                ohv = oh[:, 0:kb * 128].rearrange("p (f c) -> p c f", c=kb)
                for c in range(kb):
                    nc.tensor.matmul(
                        ps[:],
                        lhsT=ohv[:, c, :],
                        rhs=xt[:, c * XCOL:(c + 1) * XCOL],
                        start=(c == 0),
                        stop=(c == kb - 1),
                    )
                return ps

            def emit_scale(b, ps):
                # counts live in ps[:,128].  No max(count,1) guard: the host
                # guarantees every real node has count > 0 (injecting
                # 1e-30-weight phantom edges if needed); padding nodes divide
                # by zero -> NaN columns that the host slices off.
                rec = epool.tile([128, 1], f32, name=f"rec{b}", tag="rec")
                nc.vector.reciprocal(rec[:], ps[:, 128:129])
                agg = epool.tile([128, 128], bf16, name=f"agg{b}", tag="agg", bufs=5)
                nc.scalar.activation(
                    agg[:], ps[:, 0:128],
                    mybir.ActivationFunctionType.Copy, scale=rec[:, 0:1],
                )
                return agg

            def emit_tail(b, agg):
                j = b % 4
                if j == 0:
                    group_pT["t"] = psT.tile([128, 512], bf16, name=f"pT{b}", tag="pT")
                pT = group_pT["t"]
                nc.tensor.transpose(pT[:, j * 128:(j + 1) * 128], agg[:], id_t[:])
                if j == 3 or b == nblocks - 1:
                    g0 = (b // 4) * 4
                    gw = (b + 1 - g0) * 128
                    aggT = epool.tile([128, 512], bf16, name=f"aggT{b}", tag="aggT", bufs=3)
                    nc.scalar.copy(aggT[:, 0:gw], pT[:, 0:gw])
                    pO = psO.tile([128, 512], f32, name=f"pO{b}", tag="pO")
                    nc.tensor.matmul(
                        pO[:, 0:gw], lhsT=wt_t[:], rhs=aggT[:, 0:gw],
                        start=True, stop=True,
                    )
                    ot = epool.tile([128, 512], bf16, name=f"ot{b}", tag="ot", bufs=4)
                    nc.scalar.activation(
                        ot[:, 0:gw], pO[:, 0:gw],
                        mybir.ActivationFunctionType.Relu,
                        bias=bias_t[:, 0:1], scale=1.0,
                    )
                    nc.scalar.dma_start(out_ap[:, g0 * 128:(b + 1) * 128], ot[:, 0:gw])

            # Software-pipelined emission. Every engine queue is strict
            # in-order, so an op gated on *fresh* upstream state stalls the
            # whole queue behind it. Stagger each stage so, by the time a
            # queue reaches an op, its dependencies are blocks old:
            #   iter b:  DMA xe group | one-hot(b) | PE matmuls(b-1)
            #            | reciprocal+scale of (b-3) | transpose/output of (b-5)
            xt_of = {}
            gi = 0
            pending = {}
            pending_ps = {}
            pending_agg = {}
            for b in range(nblocks):
                if gi < len(groups) and b == gstart[gi]:
                    c0, c1 = prefix[gstart[gi]], prefix[gstart[gi + 1]]
                    xg = xpool.tile([128, GMAXCH * XCOL], bf16, name=f"xg{gi}", tag="xg")
                    # Alternate groups between the two hardware DMA queues
                    # (SP and ACT) so both pull HBM concurrently.
                    dma_eng = nc.sync if gi % 2 == 0 else nc.scalar
                    dma_eng.dma_start(
                        xg[:, 0:(c1 - c0) * XCOL],
                        xe_ap[:, c0 * XCOL:c1 * XCOL],
                    )
                    for bb in range(gstart[gi], gstart[gi + 1]):
                        off = (prefix[bb] - c0) * XCOL
                        xt_of[bb] = xg[:, off:off + kb_seq[bb] * XCOL]
                    gi += 1
                kb = kb_seq[b]
                oh = ohpool.tile([128, KMAX * 128], bf16, name=f"oh{b}", tag="oh")
                nc.vector.tensor_tensor(
                    out=oh[:, 0:kb * 128].rearrange("p (f c) -> p f c", c=kb),
                    in0=iotafc_t[:, ioff[kb]:ioff[kb] + kb * 128]
                        .rearrange("p (f c) -> p f c", c=kb),
                    in1=lid_t[:, prefix[b]:prefix[b] + kb]
                        .rearrange("p (o c) -> p o c", o=1)
                        .to_broadcast([128, 128, kb]),
                    op=mybir.AluOpType.is_equal,
                )
                pending[b] = oh
                if b >= 1:
                    bb = b - 1
                    pending_ps[bb] = emit_matmuls(bb, xt_of.pop(bb), pending.pop(bb))
                if b >= 3:
                    pending_agg[b - 3] = emit_scale(b - 3, pending_ps.pop(b - 3))
                if b >= 5:
                    emit_tail(b - 5, pending_agg.pop(b - 5))
            last = nblocks - 1
            pending_ps[last] = emit_matmuls(last, xt_of.pop(last), pending.pop(last))
            for bb in sorted(pending_ps):
                pending_agg[bb] = emit_scale(bb, pending_ps.pop(bb))
            for bb in sorted(pending_agg):
                emit_tail(bb, pending_agg.pop(bb))

    nc.compile()
    return nc


def _get_module(kb_seq):
    key = tuple(kb_seq)
    if key not in _module_cache:
        _module_cache[key] = _build_module(key)
    return _module_cache[key]


def prepare_inputs(edge_data, dst, W, b):
    """Host-side sharding: route each edge to the core/block owning dst."""
    edge_data = np.asarray(edge_data, dtype=np.float32)
    dst = np.asarray(dst)
    W = np.asarray(W, dtype=np.float32)
    b = np.asarray(b, dtype=np.float32)
    E = dst.shape[0]

    # The device kernel divides by the raw count (no max(count,1) guard).
    # Give any zero-degree real node a phantom edge with zero features and a
    # 1e-30 "count" weight: sums stay exactly 0, so mean = 0/1e-30 = 0, which
    # matches the reference's 0/max(0,1).
    node_cnt = np.bincount(dst, minlength=N_NODES)[:N_NODES]
    zeros = np.nonzero(node_cnt == 0)[0]
    n_real = E
    if len(zeros):
        dst = np.concatenate([dst, zeros.astype(dst.dtype)])
        E = dst.shape[0]

    blk = (dst.astype(np.int64)) >> 7                 # destination block id
    cnt = np.bincount(blk, minlength=TOTAL_BLOCKS)
    kb_all = np.maximum(1, -(-cnt // 128))            # chunks per block

    # Bin-pack: sort blocks by chunk count desc, deal round-robin to cores.
    # Every core then has the same chunk-count sequence kb_seq (per-position
    # max over cores = the first core's, since the deal preserves order).
    sortidx = np.argsort(-kb_all, kind="stable")
    core_of = np.empty(TOTAL_BLOCKS, np.int64)
    pos_of = np.empty(TOTAL_BLOCKS, np.int64)
    r = np.arange(TOTAL_BLOCKS)
    core_of[sortidx] = r % N_CORES
    pos_of[sortidx] = r // N_CORES
    kb_seq = kb_all[sortidx[0::N_CORES]]
    CH = int(kb_seq.sum())
    prefix = np.concatenate([[0], np.cumsum(kb_seq)]).astype(np.int64)

    starts = np.zeros(TOTAL_BLOCKS, np.int64)
    np.cumsum(cnt[:-1], out=starts[1:])
    order = np.argsort(blk, kind="stable")
    rank = np.empty(E, np.int64)
    rank[order] = np.arange(E, dtype=np.int64) - np.repeat(starts, cnt)

    # Flat slot in the per-core partition-major layout:
    #   (core*128 + partition) * CH + prefix[pos] + chunk
    slot = (
        (core_of[blk] * 128 + (rank & 127)) * CH
        + prefix[pos_of[blk]] + (rank >> 7)
    )

    X = np.zeros((N_CORES * 128 * CH, XCOL), BF16)
    X[slot[:n_real], 0:128] = edge_data.astype(BF16)
    X[slot[:n_real], 128] = BF16(1.0)
    if len(zeros):
        X[slot[n_real:], 128] = BF16(1e-30)
    X = X.reshape(N_CORES, 128, CH * XCOL)

    lid_f = np.full(N_CORES * 128 * CH, -1.0, np.float32)
    lid_f[slot] = (dst & 127).astype(np.float32)
    lid_all = lid_f.reshape(N_CORES, 128, CH).astype(BF16)

    wt = np.ascontiguousarray(W.T).astype(BF16)
    bias = np.ascontiguousarray(b.reshape(128, 1))
    ident = np.eye(128, dtype=np.float32).astype(BF16)
    kdistinct = sorted(set(int(k) for k in kb_seq))
    iotafc = np.concatenate(
        [np.repeat(np.arange(128, dtype=np.float32), k) for k in kdistinct]
    )
    iotafc = np.ascontiguousarray(
        np.broadcast_to(iotafc, (128, iotafc.shape[0]))
    ).astype(BF16)

    in_maps = [
        {
            "xe": np.ascontiguousarray(X[c]),
            "lid": np.ascontiguousarray(lid_all[c]),
            "wt": wt,
            "bias": bias,
            "ident": ident,
            "iotafc": iotafc,
        }
        for c in range(N_CORES)
    ]
    return kb_seq, sortidx, in_maps


def run(edge_data, dst, W, b, trace=False, tmpdir=None):
    from concourse.bass_utils import run_bass_kernel_spmd

    kb_seq, sortidx, in_maps = prepare_inputs(edge_data, dst, W, b)
    nc = _get_module(kb_seq)
    res = run_bass_kernel_spmd(
        nc, in_maps, core_ids=list(range(N_CORES)), trace=trace, tmpdir=tmpdir,
    )
    full = np.empty((TOTAL_BLOCKS * 128, 128), np.float32)
    for c in range(N_CORES):
        oc = res.results[c]["out"].T.astype(np.float32)   # [12544, 128]
        blocks = sortidx[c::N_CORES]                      # block at position j
        for j, blkid in enumerate(blocks):
            full[blkid * 128:(blkid + 1) * 128] = oc[j * 128:(j + 1) * 128]
    full = full[:N_NODES]
    return np.ascontiguousarray(full, dtype=np.float32), res


def kernel(edge_data, dst, W, b):
    out, _ = run(edge_data, dst, W, b, trace=False)
    return out


# revision 27
# speedup vs baseline: 2.2714x; 1.0153x over previous
"""GCN edge-aggregation kernel for 8 Trainium2 NeuronCores.

Math (see nn_GCNEdge): h = relu((segment_sum(edge_data, dst) / max(count,1)) @ W.T + b)

Strategy
--------
Host-side (sharding/layout only — all arithmetic happens on device):
  * Nodes live in 784 blocks of 128; each edge is routed to the block owning
    its destination node (CSR-style destination binning).  Blocks are
    bin-packed onto the 8 cores: sorted by chunk count (ceil(edges/128)) and
    dealt round-robin, so every core sees the same per-position chunk-count
    sequence kb_seq and one SPMD program serves all cores, with only ~4% slot
    padding (vs ~13% for a uniform 18-chunk capacity).
  * Edge features ship as plain bf16 (rel-err gate is 2e-2; bf16 end-to-end is
    ~3e-3), with a constant-1 count column riding along for the degree counts.
  * The xe stream is partition-major: each SBUF partition's data for a run of
    blocks is one contiguous HBM range, so multi-block DMAs move ~30KB per
    partition per transfer (big descriptors -> full HBM bandwidth).

Device-side (per core, per 128-node block):
  * One-hot matrix of local node ids (DVE is_equal against an iota pattern).
    The one-hot is laid out [partition=edge, (node f, chunk c)] — f-major with
    the chunk axis innermost — so every DVE operand has a unit-stride last
    axis, which qualifies the op for the DVE 2x_1p fast path (2 elem/cycle).
    The PE matmul then reads each chunk's one-hot with a stride-K node axis.
  * PE matmul-accumulate onehot.T @ [x | 1 | 0] into PSUM -> per-node feature
    sums and counts,
  * mean = sums * reciprocal(count) (ACT copy with per-partition scale,
    casting to bf16),
  * PE transpose (bf16), then out = relu(W @ agg.T + b) via a bf16 matmul with
    the (pre-transposed) weight as the stationary operand; output stays
    transposed [out_feat, node] in bf16 and is un-transposed / upcast on host.

No collectives are needed: output shards are disjoint.
"""

import numpy as np
import ml_dtypes

BF16 = ml_dtypes.bfloat16

N_NODES = 100000
N_EDGES = 1600000
F = 128
N_CORES = 8
BLK = 128                       # nodes per block
BLOCKS_PER_CORE = 98
TOTAL_BLOCKS = N_CORES * BLOCKS_PER_CORE        # 784
NODES_PER_CORE = BLOCKS_PER_CORE * BLK          # 12544
XCOL = 129                      # 128 features + count col

_module_cache = {}


def _make_groups():
    """Positions per xe DMA transfer: tapered head so compute starts early,
    big groups in the middle for bandwidth, tapered tail so the final blocks'
    compute overlaps the last transfers."""
    return [2, 2, 3, 4, 5] + [6] * 13 + [2, 1, 1]


def _build_module(kb_seq):
    import concourse.mybir as mybir
    import concourse.tile as tile
    from concourse import bacc

    f32 = mybir.dt.float32
    bf16 = mybir.dt.bfloat16

    kb_seq = list(kb_seq)
    nblocks = len(kb_seq)
    CH = sum(kb_seq)                       # total chunks per core
    prefix = np.concatenate([[0], np.cumsum(kb_seq)]).astype(int)
    kdistinct = sorted(set(kb_seq))
    ioff = {}
    o = 0
    for k in kdistinct:
        ioff[k] = o
        o += k * 128
    IOTA_COLS = o
    KMAX = max(kb_seq)

    groups = _make_groups()
    assert sum(groups) == nblocks
    gstart = np.concatenate([[0], np.cumsum(groups)]).astype(int)
    GMAXCH = max(
        prefix[gstart[gi + 1]] - prefix[gstart[gi]] for gi in range(len(groups))
    )

    # Greedy byte-balance the xe groups over the two hardware DMA queues
    # (SP, ACT); ACT is pre-loaded with the lid constant and the out writes.
    gbytes = [
        float(prefix[gstart[gi + 1]] - prefix[gstart[gi]]) for gi in range(len(groups))
    ]
    act_extra = (CH + nblocks * 128) / XCOL  # lid + out writes, in chunk units
    load = {0: 0.0, 1: act_extra}
    gqueue = []
    for w in gbytes:
        qsel = 0 if load[0] <= load[1] else 1
        gqueue.append(qsel)
        load[qsel] += w

    nc = bacc.Bacc("TRN2", target_bir_lowering=False, debug=False)
    xe = nc.dram_tensor("xe", [128, CH * XCOL], bf16, kind="ExternalInput")
    lid = nc.dram_tensor("lid", [128, CH], bf16, kind="ExternalInput")
    wt = nc.dram_tensor("wt", [128, 128], bf16, kind="ExternalInput")
    bias = nc.dram_tensor("bias", [128, 1], f32, kind="ExternalInput")
    ident = nc.dram_tensor("ident", [128, 128], bf16, kind="ExternalInput")
    out = nc.dram_tensor("out", [128, nblocks * 128], bf16, kind="ExternalOutput")

    xe_ap = xe.ap()
    out_ap = out.ap()

    with tile.TileContext(nc) as tc:
        with (
            tc.tile_pool(name="const", bufs=1) as cpool,
            tc.tile_pool(name="xp", bufs=5) as xpool,
            tc.tile_pool(name="ohp", bufs=8) as ohpool,
            tc.tile_pool(name="ep", bufs=3) as epool,
            tc.tile_pool(name="psS", bufs=4, space="PSUM") as psS,
            tc.tile_pool(name="psT", bufs=2, space="PSUM") as psT,
            tc.tile_pool(name="psO", bufs=2, space="PSUM") as psO,
        ):
            # Constants ride the ACT engine's hardware DMA queue so the SP
            # queue starts the bulk xe stream immediately; the one-hot
            # prerequisite (lid) loads first.  The iota compare patterns
            # (iotafc[p, ioff[k] + f*k + c] = f, one per distinct kb; values
            # 0..127 are exact in bf16) are generated on the idle GPSIMD
            # engine instead of being shipped over HBM — the pattern used by
            # the first blocks is generated first.
            lid_t = cpool.tile([128, CH], bf16)
            nc.scalar.dma_start(lid_t[:], lid.ap()[:])
            wt_t = cpool.tile([128, 128], bf16)
            nc.scalar.dma_start(wt_t[:], wt.ap()[:])
            bias_t = cpool.tile([128, 1], f32)
            nc.scalar.dma_start(bias_t[:], bias.ap()[:])
            id_t = cpool.tile([128, 128], bf16)
            nc.scalar.dma_start(id_t[:], ident.ap()[:])
            iotafc_t = cpool.tile([128, IOTA_COLS], bf16)
            for k in sorted(kdistinct, reverse=True):
                nc.gpsimd.iota(
                    iotafc_t[:, ioff[k]:ioff[k] + k * 128],
                    [[1, 128], [0, k]],
                    channel_multiplier=0,
                    allow_small_or_imprecise_dtypes=True,
                )

            group_pT = {}

            def emit_matmuls(b, xt, oh):
                kb = kb_seq[b]
                ps = psS.tile([128, XCOL], f32, name=f"ps{b}", tag="ps")
                ohv = oh[:, 0:kb * 128].rearrange("p (f c) -> p c f", c=kb)
                for c in range(kb):
                    nc.tensor.matmul(
                        ps[:],
                        lhsT=ohv[:, c, :],
                        rhs=xt[:, c * XCOL:(c + 1) * XCOL],
                        start=(c == 0),
                        stop=(c == kb - 1),
                    )
                return ps

            def emit_scale(b, ps):
                # counts live in ps[:,128].  No max(count,1) guard: the host
                # guarantees every real node has count > 0 (injecting
                # 1e-30-weight phantom edges if needed); padding nodes divide
                # by zero -> NaN columns that the host slices off.
                rec = epool.tile([128, 1], f32, name=f"rec{b}", tag="rec")
                nc.vector.reciprocal(rec[:], ps[:, 128:129])
                agg = epool.tile([128, 128], bf16, name=f"agg{b}", tag="agg", bufs=5)
                nc.scalar.activation(
                    agg[:], ps[:, 0:128],
                    mybir.ActivationFunctionType.Copy, scale=rec[:, 0:1],
                )
                return agg

            def emit_tail(b, agg):
                j = b % 4
                if j == 0:
                    group_pT["t"] = psT.tile([128, 512], bf16, name=f"pT{b}", tag="pT")
                pT = group_pT["t"]
                nc.tensor.transpose(pT[:, j * 128:(j + 1) * 128], agg[:], id_t[:])
                if j == 3 or b == nblocks - 1:
                    g0 = (b // 4) * 4
                    gw = (b + 1 - g0) * 128
                    aggT = epool.tile([128, 512], bf16, name=f"aggT{b}", tag="aggT", bufs=3)
                    nc.scalar.copy(aggT[:, 0:gw], pT[:, 0:gw])
                    pO = psO.tile([128, 512], f32, name=f"pO{b}", tag="pO")
                    nc.tensor.matmul(
                        pO[:, 0:gw], lhsT=wt_t[:], rhs=aggT[:, 0:gw],
                        start=True, stop=True,
                    )
                    ot = epool.tile([128, 512], bf16, name=f"ot{b}", tag="ot", bufs=4)
                    nc.scalar.activation(
                        ot[:, 0:gw], pO[:, 0:gw],
                        mybir.ActivationFunctionType.Relu,
                        bias=bias_t[:, 0:1], scale=1.0,
                    )
                    nc.scalar.dma_start(out_ap[:, g0 * 128:(b + 1) * 128], ot[:, 0:gw])

            # Software-pipelined emission. Every engine queue is strict
            # in-order, so an op gated on *fresh* upstream state stalls the
            # whole queue behind it. Stagger each stage so, by the time a
            # queue reaches an op, its dependencies are blocks old:
            #   iter b:  DMA xe group | one-hot(b) | PE matmuls(b-1)
            #            | reciprocal+scale of (b-3) | transpose/output of (b-5)
            xt_of = {}
            gi = 0
            pending = {}
            pending_ps = {}
            pending_agg = {}
            for b in range(nblocks):
                if gi < len(groups) and b == gstart[gi]:
                    c0, c1 = prefix[gstart[gi]], prefix[gstart[gi + 1]]
                    xg = xpool.tile([128, GMAXCH * XCOL], bf16, name=f"xg{gi}", tag="xg")
                    # Both hardware DMA queues (SP and ACT) pull HBM
                    # concurrently, byte-balanced.
                    dma_eng = nc.sync if gqueue[gi] == 0 else nc.scalar
                    dma_eng.dma_start(
                        xg[:, 0:(c1 - c0) * XCOL],
                        xe_ap[:, c0 * XCOL:c1 * XCOL],
                    )
                    for bb in range(gstart[gi], gstart[gi + 1]):
                        off = (prefix[bb] - c0) * XCOL
                        xt_of[bb] = xg[:, off:off + kb_seq[bb] * XCOL]
                    gi += 1
                kb = kb_seq[b]
                oh = ohpool.tile([128, KMAX * 128], bf16, name=f"oh{b}", tag="oh")
                oh_eng = nc.vector
                oh_eng.tensor_tensor(
                    out=oh[:, 0:kb * 128].rearrange("p (f c) -> p f c", c=kb),
                    in0=iotafc_t[:, ioff[kb]:ioff[kb] + kb * 128]
                        .rearrange("p (f c) -> p f c", c=kb),
                    in1=lid_t[:, prefix[b]:prefix[b] + kb]
                        .rearrange("p (o c) -> p o c", o=1)
                        .to_broadcast([128, 128, kb]),
                    op=mybir.AluOpType.is_equal,
                )
                pending[b] = oh
                if b >= 1:
                    bb = b - 1
                    pending_ps[bb] = emit_matmuls(bb, xt_of.pop(bb), pending.pop(bb))
                if b >= 3:
                    pending_agg[b - 3] = emit_scale(b - 3, pending_ps.pop(b - 3))
                if b >= 5:
                    emit_tail(b - 5, pending_agg.pop(b - 5))
            last = nblocks - 1
            pending_ps[last] = emit_matmuls(last, xt_of.pop(last), pending.pop(last))
            for bb in sorted(pending_ps):
                pending_agg[bb] = emit_scale(bb, pending_ps.pop(bb))
            for bb in sorted(pending_agg):
                emit_tail(bb, pending_agg.pop(bb))

    nc.compile()
    return nc


def _get_module(kb_seq):
    key = tuple(kb_seq)
    if key not in _module_cache:
        _module_cache[key] = _build_module(key)
    return _module_cache[key]


def prepare_inputs(edge_data, dst, W, b):
    """Host-side sharding: route each edge to the core/block owning dst."""
    edge_data = np.asarray(edge_data, dtype=np.float32)
    dst = np.asarray(dst)
    W = np.asarray(W, dtype=np.float32)
    b = np.asarray(b, dtype=np.float32)
    E = dst.shape[0]

    # The device kernel divides by the raw count (no max(count,1) guard).
    # Give any zero-degree real node a phantom edge with zero features and a
    # 1e-30 "count" weight: sums stay exactly 0, so mean = 0/1e-30 = 0, which
    # matches the reference's 0/max(0,1).
    node_cnt = np.bincount(dst, minlength=N_NODES)[:N_NODES]
    zeros = np.nonzero(node_cnt == 0)[0]
    n_real = E
    if len(zeros):
        dst = np.concatenate([dst, zeros.astype(dst.dtype)])
        E = dst.shape[0]

    blk = (dst.astype(np.int64)) >> 7                 # destination block id
    cnt = np.bincount(blk, minlength=TOTAL_BLOCKS)
    kb_all = np.maximum(1, -(-cnt // 128))            # chunks per block

    # Bin-pack: sort blocks by chunk count desc, deal round-robin to cores.
    # Every core then has the same chunk-count sequence kb_seq (per-position
    # max over cores = the first core's, since the deal preserves order).
    sortidx = np.argsort(-kb_all, kind="stable")
    core_of = np.empty(TOTAL_BLOCKS, np.int64)
    pos_of = np.empty(TOTAL_BLOCKS, np.int64)
    r = np.arange(TOTAL_BLOCKS)
    core_of[sortidx] = r % N_CORES
    pos_of[sortidx] = r // N_CORES
    kb_seq = kb_all[sortidx[0::N_CORES]]
    CH = int(kb_seq.sum())
    prefix = np.concatenate([[0], np.cumsum(kb_seq)]).astype(np.int64)

    starts = np.zeros(TOTAL_BLOCKS, np.int64)
    np.cumsum(cnt[:-1], out=starts[1:])
    order = np.argsort(blk, kind="stable")
    rank = np.empty(E, np.int64)
    rank[order] = np.arange(E, dtype=np.int64) - np.repeat(starts, cnt)

    # Flat slot in the per-core partition-major layout:
    #   (core*128 + partition) * CH + prefix[pos] + chunk
    slot = (
        (core_of[blk] * 128 + (rank & 127)) * CH
        + prefix[pos_of[blk]] + (rank >> 7)
    )

    X = np.zeros((N_CORES * 128 * CH, XCOL), BF16)
    X[slot[:n_real], 0:128] = edge_data.astype(BF16)
    X[slot[:n_real], 128] = BF16(1.0)
    if len(zeros):
        X[slot[n_real:], 128] = BF16(1e-30)
    X = X.reshape(N_CORES, 128, CH * XCOL)

    lid_f = np.full(N_CORES * 128 * CH, -1.0, np.float32)
    lid_f[slot] = (dst & 127).astype(np.float32)
    lid_all = lid_f.reshape(N_CORES, 128, CH).astype(BF16)

    wt = np.ascontiguousarray(W.T).astype(BF16)
    bias = np.ascontiguousarray(b.reshape(128, 1))
    ident = np.eye(128, dtype=np.float32).astype(BF16)

    in_maps = [
        {
            "xe": np.ascontiguousarray(X[c]),
            "lid": np.ascontiguousarray(lid_all[c]),
            "wt": wt,
            "bias": bias,
            "ident": ident,
        }
        for c in range(N_CORES)
    ]
    return kb_seq, sortidx, in_maps


def run(edge_data, dst, W, b, trace=False, tmpdir=None):
    from concourse.bass_utils import run_bass_kernel_spmd

    kb_seq, sortidx, in_maps = prepare_inputs(edge_data, dst, W, b)
    nc = _get_module(kb_seq)
    res = run_bass_kernel_spmd(
        nc, in_maps, core_ids=list(range(N_CORES)), trace=trace, tmpdir=tmpdir,
    )
    full = np.empty((TOTAL_BLOCKS * 128, 128), np.float32)
    for c in range(N_CORES):
        oc = res.results[c]["out"].T.astype(np.float32)   # [12544, 128]
        blocks = sortidx[c::N_CORES]                      # block at position j
        for j, blkid in enumerate(blocks):
            full[blkid * 128:(blkid + 1) * 128] = oc[j * 128:(j + 1) * 128]
    full = full[:N_NODES]
    return np.ascontiguousarray(full, dtype=np.float32), res


def kernel(edge_data, dst, W, b):
    out, _ = run(edge_data, dst, W, b, trace=False)
    return out


# revision 28
# speedup vs baseline: 2.5314x; 1.1145x over previous
"""GCN edge-aggregation kernel for 8 Trainium2 NeuronCores.

Math (see nn_GCNEdge): h = relu((segment_sum(edge_data, dst) / max(count,1)) @ W.T + b)

Strategy
--------
Host-side (sharding/layout only — all arithmetic happens on device):
  * Nodes live in 784 blocks of 128; each edge is routed to the block owning
    its destination node (CSR-style destination binning).  Blocks are
    bin-packed onto the 8 cores: sorted by chunk count (ceil(edges/128)) and
    dealt round-robin, so every core sees the same per-position chunk-count
    sequence kb_seq and one SPMD program serves all cores, with only ~4% slot
    padding (vs ~13% for a uniform 18-chunk capacity).
  * Edge features ship as fp8 e3m4 (rel-err gate is 2e-2; e3m4 end-to-end is
    ~1.5e-2), with a constant-1 count column riding along for the degree counts.
  * The xe stream is partition-major: each SBUF partition's data for a run of
    blocks is one contiguous HBM range, so multi-block DMAs move ~30KB per
    partition per transfer (big descriptors -> full HBM bandwidth).

Device-side (per core, per 128-node block):
  * One-hot matrix of local node ids (DVE is_equal against an iota pattern).
    The one-hot is laid out [partition=edge, (node f, chunk c)] — f-major with
    the chunk axis innermost — so every DVE operand has a unit-stride last
    axis, which qualifies the op for the DVE 2x_1p fast path (2 elem/cycle).
    The PE matmul then reads each chunk's one-hot with a stride-K node axis.
  * PE matmul-accumulate onehot.T @ [x | 1 | 0] into PSUM -> per-node feature
    sums and counts,
  * mean = sums * reciprocal(count) (ACT copy with per-partition scale,
    casting to bf16),
  * PE transpose (bf16), then out = relu(W @ agg.T + b) via a bf16 matmul with
    the (pre-transposed) weight as the stationary operand; output stays
    transposed [out_feat, node] in bf16 and is un-transposed / upcast on host.

No collectives are needed: output shards are disjoint.
"""

import numpy as np
import ml_dtypes

BF16 = ml_dtypes.bfloat16

N_NODES = 100000
N_EDGES = 1600000
F = 128
N_CORES = 8
BLK = 128                       # nodes per block
BLOCKS_PER_CORE = 98
TOTAL_BLOCKS = N_CORES * BLOCKS_PER_CORE        # 784
NODES_PER_CORE = BLOCKS_PER_CORE * BLK          # 12544
XCOL = 129                      # 128 features + count col

_module_cache = {}


def _make_groups():
    """Positions per xe DMA transfer: tapered head so compute starts early,
    big groups in the middle for bandwidth, tapered tail so the final blocks'
    compute overlaps the last transfers."""
    return [2, 2, 3, 4, 5] + [6] * 13 + [2, 1, 1]


def _build_module(kb_seq):
    import concourse.mybir as mybir
    import concourse.tile as tile
    from concourse import bacc

    f32 = mybir.dt.float32
    bf16 = mybir.dt.bfloat16
    fp8 = mybir.dt.float8e3

    kb_seq = list(kb_seq)
    nblocks = len(kb_seq)
    CH = sum(kb_seq)                       # total chunks per core
    prefix = np.concatenate([[0], np.cumsum(kb_seq)]).astype(int)
    kdistinct = sorted(set(kb_seq))
    ioff = {}
    o = 0
    for k in kdistinct:
        ioff[k] = o
        o += k * 128
    IOTA_COLS = o
    KMAX = max(kb_seq)

    groups = _make_groups()
    assert sum(groups) == nblocks
    gstart = np.concatenate([[0], np.cumsum(groups)]).astype(int)
    GMAXCH = max(
        prefix[gstart[gi + 1]] - prefix[gstart[gi]] for gi in range(len(groups))
    )

    # Greedy byte-balance the xe groups over the two hardware DMA queues
    # (SP, ACT); ACT is pre-loaded with the lid constant and the out writes.
    gbytes = [
        float(prefix[gstart[gi + 1]] - prefix[gstart[gi]]) for gi in range(len(groups))
    ]
    act_extra = 2 * (CH + nblocks * 128) / XCOL  # lid + out writes (2B/elem), in xe-chunk units
    load = {0: 0.0, 1: act_extra}
    gqueue = []
    for w in gbytes:
        qsel = 0 if load[0] <= load[1] else 1
        gqueue.append(qsel)
        load[qsel] += w

    nc = bacc.Bacc("TRN2", target_bir_lowering=False, debug=False)
    xe = nc.dram_tensor("xe", [128, CH * XCOL], fp8, kind="ExternalInput")
    lid = nc.dram_tensor("lid", [128, CH], bf16, kind="ExternalInput")
    wt = nc.dram_tensor("wt", [128, 128], bf16, kind="ExternalInput")
    bias = nc.dram_tensor("bias", [128, 1], f32, kind="ExternalInput")
    ident = nc.dram_tensor("ident", [128, 128], bf16, kind="ExternalInput")
    out = nc.dram_tensor("out", [128, nblocks * 128], bf16, kind="ExternalOutput")

    xe_ap = xe.ap()
    out_ap = out.ap()

    with tile.TileContext(nc) as tc:
        with (
            tc.tile_pool(name="const", bufs=1) as cpool,
            tc.tile_pool(name="xp", bufs=5) as xpool,
            tc.tile_pool(name="ohp", bufs=8) as ohpool,
            tc.tile_pool(name="ep", bufs=3) as epool,
            tc.tile_pool(name="psS", bufs=4, space="PSUM") as psS,
            tc.tile_pool(name="psT", bufs=2, space="PSUM") as psT,
            tc.tile_pool(name="psO", bufs=2, space="PSUM") as psO,
        ):
            # Constants ride the ACT engine's hardware DMA queue so the SP
            # queue starts the bulk xe stream immediately; the one-hot
            # prerequisite (lid) loads first.  The iota compare patterns
            # (iotafc[p, ioff[k] + f*k + c] = f, one per distinct kb; values
            # 0..127 are exact in bf16) are generated on the idle GPSIMD
            # engine instead of being shipped over HBM — the pattern used by
            # the first blocks is generated first.
            lid_t = cpool.tile([128, CH], bf16)
            nc.scalar.dma_start(lid_t[:], lid.ap()[:])
            wt_t = cpool.tile([128, 128], bf16)
            nc.scalar.dma_start(wt_t[:], wt.ap()[:])
            bias_t = cpool.tile([128, 1], f32)
            nc.scalar.dma_start(bias_t[:], bias.ap()[:])
            id_t = cpool.tile([128, 128], bf16)
            nc.scalar.dma_start(id_t[:], ident.ap()[:])
            iotafc_t = cpool.tile([128, IOTA_COLS], bf16)
            for k in sorted(kdistinct, reverse=True):
                nc.gpsimd.iota(
                    iotafc_t[:, ioff[k]:ioff[k] + k * 128],
                    [[1, 128], [0, k]],
                    channel_multiplier=0,
                    allow_small_or_imprecise_dtypes=True,
                )

            group_pT = {}

            def emit_matmuls(b, xt, oh):
                kb = kb_seq[b]
                ps = psS.tile([128, XCOL], f32, name=f"ps{b}", tag="ps")
                ohv = oh[:, 0:kb * 128].rearrange("p (f c) -> p c f", c=kb)
                for c in range(kb):
                    nc.tensor.matmul(
                        ps[:],
                        lhsT=ohv[:, c, :],
                        rhs=xt[:, c * XCOL:(c + 1) * XCOL],
                        start=(c == 0),
                        stop=(c == kb - 1),
                    )
                return ps

            def emit_scale(b, ps):
                # counts live in ps[:,128].  No max(count,1) guard: the host
                # guarantees every real node has count > 0 (injecting
                # zero-feature phantom edges if needed); padding nodes divide
                # by zero -> NaN columns that the host slices off.
                rec = epool.tile([128, 1], f32, name=f"rec{b}", tag="rec")
                nc.vector.reciprocal(rec[:], ps[:, 128:129])
                agg = epool.tile([128, 128], bf16, name=f"agg{b}", tag="agg", bufs=5)
                nc.scalar.activation(
                    agg[:], ps[:, 0:128],
                    mybir.ActivationFunctionType.Copy, scale=rec[:, 0:1],
                )
                return agg

            def emit_tail(b, agg):
                j = b % 4
                if j == 0:
                    group_pT["t"] = psT.tile([128, 512], bf16, name=f"pT{b}", tag="pT")
                pT = group_pT["t"]
                nc.tensor.transpose(pT[:, j * 128:(j + 1) * 128], agg[:], id_t[:])
                if j == 3 or b == nblocks - 1:
                    g0 = (b // 4) * 4
                    gw = (b + 1 - g0) * 128
                    aggT = epool.tile([128, 512], bf16, name=f"aggT{b}", tag="aggT", bufs=3)
                    nc.scalar.copy(aggT[:, 0:gw], pT[:, 0:gw])
                    pO = psO.tile([128, 512], f32, name=f"pO{b}", tag="pO")
                    nc.tensor.matmul(
                        pO[:, 0:gw], lhsT=wt_t[:], rhs=aggT[:, 0:gw],
                        start=True, stop=True,
                    )
                    ot = epool.tile([128, 512], bf16, name=f"ot{b}", tag="ot", bufs=4)
                    nc.scalar.activation(
                        ot[:, 0:gw], pO[:, 0:gw],
                        mybir.ActivationFunctionType.Relu,
                        bias=bias_t[:, 0:1], scale=1.0,
                    )
                    nc.scalar.dma_start(out_ap[:, g0 * 128:(b + 1) * 128], ot[:, 0:gw])

            # Software-pipelined emission. Every engine queue is strict
            # in-order, so an op gated on *fresh* upstream state stalls the
            # whole queue behind it. Stagger each stage so, by the time a
            # queue reaches an op, its dependencies are blocks old:
            #   iter b:  DMA xe group | one-hot(b) | PE matmuls(b-1)
            #            | reciprocal+scale of (b-3) | transpose/output of (b-5)
            xt_of = {}
            gi = 0
            pending = {}
            pending_ps = {}
            pending_agg = {}
            for b in range(nblocks):
                if gi < len(groups) and b == gstart[gi]:
                    c0, c1 = prefix[gstart[gi]], prefix[gstart[gi + 1]]
                    xg = xpool.tile([128, GMAXCH * XCOL], fp8, name=f"xg{gi}", tag="xg")
                    # Both hardware DMA queues (SP and ACT) pull HBM
                    # concurrently, byte-balanced.
                    dma_eng = nc.sync if gqueue[gi] == 0 else nc.scalar
                    dma_eng.dma_start(
                        xg[:, 0:(c1 - c0) * XCOL],
                        xe_ap[:, c0 * XCOL:c1 * XCOL],
                    )
                    for bb in range(gstart[gi], gstart[gi + 1]):
                        off = (prefix[bb] - c0) * XCOL
                        xt_of[bb] = xg[:, off:off + kb_seq[bb] * XCOL]
                    gi += 1
                kb = kb_seq[b]
                oh = ohpool.tile([128, KMAX * 128], bf16, name=f"oh{b}", tag="oh")
                oh_eng = nc.vector
                oh_eng.tensor_tensor(
                    out=oh[:, 0:kb * 128].rearrange("p (f c) -> p f c", c=kb),
                    in0=iotafc_t[:, ioff[kb]:ioff[kb] + kb * 128]
                        .rearrange("p (f c) -> p f c", c=kb),
                    in1=lid_t[:, prefix[b]:prefix[b] + kb]
                        .rearrange("p (o c) -> p o c", o=1)
                        .to_broadcast([128, 128, kb]),
                    op=mybir.AluOpType.is_equal,
                )
                pending[b] = oh
                if b >= 1:
                    bb = b - 1
                    pending_ps[bb] = emit_matmuls(bb, xt_of.pop(bb), pending.pop(bb))
                if b >= 3:
                    pending_agg[b - 3] = emit_scale(b - 3, pending_ps.pop(b - 3))
                if b >= 5:
                    emit_tail(b - 5, pending_agg.pop(b - 5))
            last = nblocks - 1
            pending_ps[last] = emit_matmuls(last, xt_of.pop(last), pending.pop(last))
            for bb in sorted(pending_ps):
                pending_agg[bb] = emit_scale(bb, pending_ps.pop(bb))
            for bb in sorted(pending_agg):
                emit_tail(bb, pending_agg.pop(bb))

    nc.compile()
    return nc


def _get_module(kb_seq):
    key = tuple(kb_seq)
    if key not in _module_cache:
        _module_cache[key] = _build_module(key)
    return _module_cache[key]


def prepare_inputs(edge_data, dst, W, b):
    """Host-side sharding: route each edge to the core/block owning dst."""
    edge_data = np.asarray(edge_data, dtype=np.float32)
    dst = np.asarray(dst)
    W = np.asarray(W, dtype=np.float32)
    b = np.asarray(b, dtype=np.float32)
    E = dst.shape[0]

    # The device kernel divides by the raw count (no max(count,1) guard).
    # Give any zero-degree real node a phantom edge with zero features and a
    # unit count weight: sums stay exactly 0, so mean = 0/1 = 0, which
    # matches the reference's 0/max(0,1).
    node_cnt = np.bincount(dst, minlength=N_NODES)[:N_NODES]
    zeros = np.nonzero(node_cnt == 0)[0]
    n_real = E
    if len(zeros):
        dst = np.concatenate([dst, zeros.astype(dst.dtype)])
        E = dst.shape[0]

    blk = (dst.astype(np.int64)) >> 7                 # destination block id
    cnt = np.bincount(blk, minlength=TOTAL_BLOCKS)
    kb_all = np.maximum(1, -(-cnt // 128))            # chunks per block

    # Bin-pack: sort blocks by chunk count desc, deal round-robin to cores.
    # Every core then has the same chunk-count sequence kb_seq (per-position
    # max over cores = the first core's, since the deal preserves order).
    sortidx = np.argsort(-kb_all, kind="stable")
    core_of = np.empty(TOTAL_BLOCKS, np.int64)
    pos_of = np.empty(TOTAL_BLOCKS, np.int64)
    r = np.arange(TOTAL_BLOCKS)
    core_of[sortidx] = r % N_CORES
    pos_of[sortidx] = r // N_CORES
    kb_seq = kb_all[sortidx[0::N_CORES]]
    CH = int(kb_seq.sum())
    prefix = np.concatenate([[0], np.cumsum(kb_seq)]).astype(np.int64)

    starts = np.zeros(TOTAL_BLOCKS, np.int64)
    np.cumsum(cnt[:-1], out=starts[1:])
    order = np.argsort(blk, kind="stable")
    rank = np.empty(E, np.int64)
    rank[order] = np.arange(E, dtype=np.int64) - np.repeat(starts, cnt)

    # Flat slot in the per-core partition-major layout:
    #   (core*128 + partition) * CH + prefix[pos] + chunk
    slot = (
        (core_of[blk] * 128 + (rank & 127)) * CH
        + prefix[pos_of[blk]] + (rank >> 7)
    )

    FP8 = ml_dtypes.float8_e3m4
    X = np.zeros((N_CORES * 128 * CH, XCOL), FP8)
    X[slot[:n_real], 0:128] = edge_data.astype(FP8)
    X[slot[:n_real], 128] = FP8(1.0)
    if len(zeros):
        X[slot[n_real:], 128] = FP8(1.0)
    X = X.reshape(N_CORES, 128, CH * XCOL)

    lid_f = np.full(N_CORES * 128 * CH, -1.0, np.float32)
    lid_f[slot] = (dst & 127).astype(np.float32)
    lid_all = lid_f.reshape(N_CORES, 128, CH).astype(BF16)

    wt = np.ascontiguousarray(W.T).astype(BF16)
    bias = np.ascontiguousarray(b.reshape(128, 1))
    ident = np.eye(128, dtype=np.float32).astype(BF16)

    in_maps = [
        {
            "xe": np.ascontiguousarray(X[c]),
            "lid": np.ascontiguousarray(lid_all[c]),
            "wt": wt,
            "bias": bias,
            "ident": ident,
        }
        for c in range(N_CORES)
    ]
    return kb_seq, sortidx, in_maps


def run(edge_data, dst, W, b, trace=False, tmpdir=None):
    from concourse.bass_utils import run_bass_kernel_spmd

    kb_seq, sortidx, in_maps = prepare_inputs(edge_data, dst, W, b)
    nc = _get_module(kb_seq)
    res = run_bass_kernel_spmd(
        nc, in_maps, core_ids=list(range(N_CORES)), trace=trace, tmpdir=tmpdir,
    )
    full = np.empty((TOTAL_BLOCKS * 128, 128), np.float32)
    for c in range(N_CORES):
        oc = res.results[c]["out"].T.astype(np.float32)   # [12544, 128]
        blocks = sortidx[c::N_CORES]                      # block at position j
        for j, blkid in enumerate(blocks):
            full[blkid * 128:(blkid + 1) * 128] = oc[j * 128:(j + 1) * 128]
    full = full[:N_NODES]
    return np.ascontiguousarray(full, dtype=np.float32), res


def kernel(edge_data, dst, W, b):
    out, _ = run(edge_data, dst, W, b, trace=False)
    return out
